# revision 1
# baseline (speedup 1.0000x reference)
"""Linear attention (ELU+1 feature map) on 8 TRN2 NeuronCores.

Reference math (per batch b):
    q,k,v = x @ W{q,k,v}.T + b;   q,k -> elu(.)+1
    kv[h,d,e] = sum_t k[t,h,d] v[t,h,e];   ks[h,d] = sum_t k[t,h,d]
    out = ((q kv) / clip(q . ks, 1e-6)) @ Wo.T + bo

Sharding: the 16384 tokens are split into 8 contiguous chunks of 2048; core c
owns batch c//2, T-half c%2. kv/ks are partial sums over the core's tokens,
AllReduce-summed within core pairs {0,1},{2,3},{4,5},{6,7} (one pair = one
batch, adjacent NeuronCores). Everything else is embarrassingly parallel, so
the only communication is a 520 KB pair AllReduce that overlaps the q
projection.

Per-core device program (S=2048 local tokens; a "pair" p = 2 heads = 128
channels; all layouts chosen so no on-device transposes are ever needed):
  phase 1: k,v projections in token-major layout via matmul(lhsT=xT block,
           rhs=W.T stripe). v is written into an interleaved pair layout with
           two ones-columns per pair (single strided 3D-AP copy per PSUM
           chunk), so ONE N=130 matmul per (pair, token-tile) produces both
           the kv outer-product block and the k-sum column, feature-major.
           Partial kv products are DVE-accumulated into SBUF (PSUM
           accumulation groups interleaved within a bank lose their first
           contribution on HW - a sibling group's start=True clears the
           bank's has_written bits).
  phase 2: qT feature-major via matmul(lhsT=Wq.T columns, rhs=xT);
           denominators via a block-diagonal ksum matmul, batched along the
           free dim so one DVE reciprocal serves all pairs; 1/denom is
           broadcast across partitions with a tiny [2,128] selector matmul;
           att = num * (1/denom) stays feature-major and feeds the output
           projection as its stationary operand; y lands token-major in PSUM
           and is copied out through SBUF.

COMPUTE selects the TensorEngine dtype: "f32r" (default) stores f32 bits and
runs the PE in round-trip fp32 mode (full rate at N>=256; ~3.5e-4 rel err),
"bf16" halves the DMA/SBUF footprint (~5.6e-3 rel err), "f32" is the exact
but 4x-slower fallback. Inputs are pre-transposed/sharded on the host; biases
are folded in via an extra ones-row contraction tile only when nonzero (the
bo bias is applied on the host).
"""

import sys
import numpy as np

for _p in ("/opt/trn_rl_repo", "/opt/pypackages"):
    if _p not in sys.path:
        sys.path.append(_p)

import concourse.bacc as bacc
import concourse.mybir as mybir
import concourse.tile as tile
from concourse import bass_utils

F32 = mybir.dt.float32
ACTF = mybir.ActivationFunctionType

N_CORES = 8
B, T, C = 4, 4096, 1024
H, D = 16, 64
S = B * T // N_CORES          # 2048 tokens per core
NP = 8                        # head pairs (128 channels each)
TT = S // 128                 # 16 token tiles per core
HALF = S // 2                 # phase-2 half size (1024)
PSTR = 130                    # kv_aug per-pair stride: 128 kv cols + ksum col
                              # + pad col (f32r matmul needs even N)

COMPUTE = "f32r"              # "f32r" | "bf16" | "f32"
DEBUG_DUMPS = False
REPEAT = 1                    # timing only: emit the body N times in one NEFF

_cache = {}


def _cdt():
    """Storage dtype of matmul-feeding tensors (f32r is f32 bits; the PE
    runs it at full rate when N>=256 and the verifier requires producers
    to declare the f32r dtype end-to-end)."""
    return {"bf16": mybir.dt.bfloat16,
            "f32r": mybir.dt.float32r,
            "f32": F32}[COMPUTE]


def _mm(ap):
    return ap


def _msview(ap):
    """Memset target view: walrus rejects Memset on f32r APs, so write the
    same bits through an f32 view."""
    return ap.bitcast(F32) if COMPUTE == "f32r" else ap


def _emit(nc, tc, KT, xt_d, wk_d, wv_d, wq_d, wo_d, cs_d, out_d, dbg=None):
    dbg = dbg or {}
    cdt = _cdt()
    res_xt = COMPUTE == "bf16"   # 2-byte xT fits SBUF for both phases
    span = HALF if COMPUTE == "bf16" else S // 4
    nchk = span // 512
    Relu, Exp = ACTF.Relu, ACTF.Exp
    WB = 2 * KT  # weight slots: wk+wv in phase 1, reused by wq+wo in phase 2

    with (
        tc.tile_pool(name="wpool", bufs=1) as wpool,
        tc.tile_pool(name="persist", bufs=1) as sb,
        tc.tile_pool(name="dram", bufs=1, space="DRAM") as dram,
    ):
        wk_sb = []
        wv_sb = []
        for ct in range(KT):
            w = wpool.tile([128, C], cdt, tag="w", bufs=WB, name=f"wk{ct}")
            nc.gpsimd.dma_start(w[:], wk_d[ct * 128:(ct + 1) * 128, :])
            wk_sb.append(w)
        for ct in range(KT):
            w = wpool.tile([128, C], cdt, tag="w", bufs=WB, name=f"wv{ct}")
            nc.gpsimd.dma_start(w[:], wv_d[ct * 128:(ct + 1) * 128, :])
            wv_sb.append(w)

        csel = sb.tile([2, 128], cdt, tag="csel", name="csel")
        nc.sync.dma_start(csel[:], cs_d[:])

        kvagg = sb.tile([128, NP * PSTR], F32, tag="kvagg", name="kvagg")

        # ------------- phase 1: k/v projections + kv aggregation -------------
        # NOTE: PSUM accumulation groups interleaved within one bank are
        # broken on HW (a sibling group's start=True clears the bank's
        # has_written bits), so kv partial products are single-shot matmuls
        # accumulated into SBUF by the DVE instead.
        with (
            tc.tile_pool(name="p1sb", bufs=1) as p1,
            tc.tile_pool(name="p1ps", bufs=1, space="PSUM") as ps1,
        ):
            nc.gpsimd.memset(kvagg[:], 0.0)

            # xT stripes: one efficient full-row DMA each (the per-token-tile
            # [128,128] block loads were 512 B/line descriptor-dominated).
            # bf16: allocated from the persistent pool and reused in phase 2.
            xs_pool = sb if res_xt else p1
            xs_sb = []
            for ct in range(KT):
                xst = xs_pool.tile([128, S], cdt, tag="xs", bufs=KT,
                                   name=f"xs{ct}")
                nc.gpsimd.dma_start(xst[:], xt_d[ct * 128:(ct + 1) * 128, :])
                xs_sb.append(xst)

            for tt in range(TT):
                t0 = tt * 128
                xb = [xs_sb[ct][:, t0:t0 + 128] for ct in range(KT)]

                ktok = p1.tile([128, C], cdt, tag="ktok", bufs=3,
                               name=f"ktok{tt}")
                kps, t1s, t2s = [], [], []
                for ch in range(2):
                    kp = ps1.tile([128, 512], F32, tag="ps", bufs=4,
                                  name=f"kp{tt}_{ch}")
                    for ct in range(KT):
                        nc.tensor.matmul(
                            kp[:], _mm(xb[ct]),
                            _mm(wk_sb[ct][:, ch * 512:(ch + 1) * 512]),
                            start=(ct == 0), stop=(ct == KT - 1))
                    kps.append(kp)
                    t1s.append(p1.tile([128, 512], F32, tag="t1", bufs=3,
                                       name=f"t1_{tt}_{ch}"))
                    t2s.append(p1.tile([128, 512], F32, tag="t2", bufs=3,
                                       name=f"t2_{tt}_{ch}"))
                # group by ACT function to avoid per-op table swaps
                for ch in range(2):
                    ks = ktok[:, ch * 512:(ch + 1) * 512]
                    nc.scalar.activation(ks, kps[ch][:], Relu)
                    nc.scalar.activation(t1s[ch][:], kps[ch][:], Relu,
                                         scale=-1.0)
                for ch in range(2):
                    nc.scalar.activation(t2s[ch][:], t1s[ch][:], Exp,
                                         scale=-1.0)
                for ch in range(2):
                    ks = ktok[:, ch * 512:(ch + 1) * 512]
                    nc.vector.tensor_add(ks, ks, t2s[ch][:])

                # v in interleaved pair layout [.. 128 v cols | 2 ones ..]
                # so one N=130 matmul per pair yields kv plus the k-sum.
                # Ones come from a whole-tile memset; v lands via ONE strided
                # 3D-AP copy per psum chunk.
                vaug = p1.tile([128, NP * PSTR], cdt, tag="vaug", bufs=3,
                               name=f"vaug{tt}")
                nc.gpsimd.memset(_msview(vaug[:]), 1.0)
                vau3 = vaug.rearrange("p (g c) -> p g c", c=PSTR)
                for ch in range(2):
                    vp = ps1.tile([128, 512], F32, tag="ps", bufs=4,
                                  name=f"vp{tt}_{ch}")
                    for ct in range(KT):
                        nc.tensor.matmul(
                            vp[:], _mm(xb[ct]),
                            _mm(wv_sb[ct][:, ch * 512:(ch + 1) * 512]),
                            start=(ct == 0), stop=(ct == KT - 1))
                    nc.vector.tensor_copy(
                        vau3[:, ch * 4:(ch + 1) * 4, 0:128],
                        vp[:].rearrange("p (g c) -> p g c", c=128))

                for g in range(3):
                    p0, p1n = 3 * g, min(3 * g + 3, NP)
                    kvt = ps1.tile([128, (p1n - p0) * PSTR], F32, tag="kvt",
                                   bufs=3, name=f"kvt{tt}_{g}",
                                   padded_shape=[128, 3 * PSTR])
                    for p in range(p0, p1n):
                        j = p - p0
                        nc.tensor.matmul(
                            kvt[:, j * PSTR:(j + 1) * PSTR],
                            _mm(ktok[:, p * 128:(p + 1) * 128]),
                            _mm(vaug[:, p * PSTR:(p + 1) * PSTR]),
                            start=True, stop=True)
                    nc.vector.tensor_add(
                        kvagg[:, p0 * PSTR:p1n * PSTR],
                        kvagg[:, p0 * PSTR:p1n * PSTR], kvt[:])

                if tt == 0 and "ktok0" in dbg:
                    kd = p1.tile([128, C], F32, tag="ktd", name="ktd")
                    nc.vector.tensor_copy(kd[:], ktok[:])
                    nc.sync.dma_start(dbg["ktok0"][:], kd[:])
                    vd = p1.tile([128, C], F32, tag="vtd", name="vtd")
                    nc.vector.tensor_copy(vd[:], vtok[:])
                    nc.sync.dma_start(dbg["vtok0"][:], vd[:])


        # ------------- pair AllReduce ----------------------------------------
        bounce_in = dram.tile([128, NP * PSTR], F32, name="bounce_in")
        bounce_out = dram.tile([128, NP * PSTR], F32, name="bounce_out")
        nc.sync.dma_start(bounce_in[:], kvagg[:])
        nc.gpsimd.collective_compute(
            "AllReduce", mybir.AluOpType.add,
            ins=[bounce_in.opt()], outs=[bounce_out.opt()],
            replica_groups=[[2 * i, 2 * i + 1] for i in range(N_CORES // 2)])
        kvcoll = sb.tile([128, NP * PSTR], F32, tag="kvcoll", name="kvcoll")
        nc.sync.dma_start(kvcoll[:], bounce_out[:])
        if "kvcoll" in dbg:
            nc.sync.dma_start(dbg["kvcoll"][:], kvcoll[:])
            nc.sync.dma_start(dbg["kvagg"][:], kvagg[:])

        # phase-2 weights (reuse the phase-1 weight slots)
        wq_sb = []
        wo_sb = []
        for ct in range(KT):
            w = wpool.tile([128, C], cdt, tag="w", bufs=WB, name=f"wq{ct}")
            nc.gpsimd.dma_start(w[:], wq_d[ct * 128:(ct + 1) * 128, :])
            wq_sb.append(w)
        for ct in range(NP):
            w = wpool.tile([128, C], cdt, tag="w", bufs=WB, name=f"wo{ct}")
            nc.gpsimd.dma_start(w[:], wo_d[ct * 128:(ct + 1) * 128, :])
            wo_sb.append(w)

        # block-diagonal kv (cross-head junk zeroed) + ksum column tiles
        kvblk = []
        ksb = []
        for p in range(NP):
            c0 = p * PSTR
            kb = sb.tile([128, 128], cdt, tag="kvblk", bufs=NP,
                         name=f"kvblk{p}")
            nc.gpsimd.memset(_msview(kb[:]), 0.0)
            nc.vector.tensor_copy(kb[0:64, 0:64], kvcoll[0:64, c0:c0 + 64])
            nc.vector.tensor_copy(kb[64:128, 64:128],
                                  kvcoll[64:128, c0 + 64:c0 + 128])
            kvblk.append(kb)
            kt = sb.tile([128, 2], cdt, tag="ksb", bufs=NP, name=f"ksb{p}")
            nc.gpsimd.memset(_msview(kt[:]), 0.0)
            nc.vector.tensor_copy(kt[0:64, 0:1],
                                  kvcoll[0:64, c0 + 128:c0 + 129])
            nc.vector.tensor_copy(kt[64:128, 1:2],
                                  kvcoll[64:128, c0 + 128:c0 + 129])
            ksb.append(kt)

        # ------------- phase 2: q, attention, output projection --------------
        with (
            tc.tile_pool(name="p2sb", bufs=1) as p2,
            tc.tile_pool(name="p2ps", bufs=1, space="PSUM") as ps2,
        ):
            for hv in range(S // span):
                hb = hv * span
                if res_xt:
                    xh = [xs_sb[ct][:, hb:hb + span] for ct in range(KT)]
                else:
                    xh = []
                    for ct in range(KT):
                        xht = p2.tile([128, span], cdt, tag="xh",
                                      bufs=KT + 1, name=f"xh{hv}_{ct}")
                        nc.sync.dma_start(
                            xht[:],
                            xt_d[ct * 128:(ct + 1) * 128, hb:hb + span])
                        xh.append(xht)

                dnb = p2.tile([2, NP * span], F32, tag="dnb", bufs=1,
                              name=f"dnb{hv}")
                qts = []
                for p in range(NP):
                    qt = p2.tile([128, span], cdt, tag="qt", bufs=NP + 1,
                                 name=f"qt{hv}_{p}")
                    qts.append(qt)
                    qps, t1s, t2s = [], [], []
                    for chk in range(nchk):
                        qp = ps2.tile([128, 512], F32, tag="ps", bufs=6,
                                      name=f"qp{hv}_{p}_{chk}")
                        for ct in range(KT):
                            nc.tensor.matmul(
                                qp[:],
                                _mm(wq_sb[ct][:, p * 128:(p + 1) * 128]),
                                _mm(xh[ct][:, chk * 512:(chk + 1) * 512]
                                    if not res_xt else
                                    xs_sb[ct][:, hb + chk * 512:
                                              hb + (chk + 1) * 512]),
                                start=(ct == 0), stop=(ct == KT - 1))
                        qps.append(qp)
                        t1s.append(p2.tile([128, 512], F32, tag="qt1",
                                           bufs=3, name=f"qt1_{hv}_{p}_{chk}"))
                        t2s.append(p2.tile([128, 512], F32, tag="qt2",
                                           bufs=3, name=f"qt2_{hv}_{p}_{chk}"))
                    for chk in range(nchk):
                        qs = qt[:, chk * 512:(chk + 1) * 512]
                        nc.scalar.activation(qs, qps[chk][:], Relu)
                        nc.scalar.activation(t1s[chk][:], qps[chk][:], Relu,
                                             scale=-1.0)
                    for chk in range(nchk):
                        nc.scalar.activation(t2s[chk][:], t1s[chk][:], Exp,
                                             scale=-1.0)
                    for chk in range(nchk):
                        qs = qt[:, chk * 512:(chk + 1) * 512]
                        nc.vector.tensor_add(qs, qs, t2s[chk][:])

                    for chk in range(nchk):
                        dn = ps2.tile([2, 512], F32, tag="dn", bufs=2,
                                      name=f"dn{hv}_{p}_{chk}")
                        nc.tensor.matmul(
                            dn[:], _mm(ksb[p][:]),
                            _mm(qt[:, chk * 512:(chk + 1) * 512]),
                            start=True, stop=True)
                        nc.vector.tensor_scalar_max(
                            dnb[:, p * span + chk * 512:
                                p * span + (chk + 1) * 512], dn[:], 1e-6)

                recb = p2.tile([2, NP * span], cdt, tag="recb", bufs=1,
                               name=f"recb{hv}")
                with nc.allow_low_precision(reason="recip of clipped denom"):
                    nc.vector.reciprocal(recb[:], dnb[:])

                att = []
                for p in range(NP):
                    qt = qts[p]
                    at = p2.tile([128, span], cdt, tag="att", bufs=NP + 1,
                                 name=f"att{hv}_{p}")
                    for chk in range(nchk):
                        nm = ps2.tile([128, 512], F32, tag="ps", bufs=6,
                                      name=f"nm{hv}_{p}_{chk}")
                        nc.tensor.matmul(
                            nm[:], _mm(kvblk[p][:]),
                            _mm(qt[:, chk * 512:(chk + 1) * 512]),
                            start=True, stop=True)
                        rp = ps2.tile([128, 512], F32, tag="ps", bufs=6,
                                      name=f"rp{hv}_{p}_{chk}")
                        nc.tensor.matmul(
                            rp[:], _mm(csel[:]),
                            _mm(recb[:, p * span + chk * 512:
                                     p * span + (chk + 1) * 512]),
                            start=True, stop=True)
                        ats = at[:, chk * 512:(chk + 1) * 512]
                        nc.scalar.copy(ats, nm[:])
                        nc.vector.tensor_mul(ats, ats, rp[:])
                    att.append(at)

                for mt in range(span // 128):
                    r0 = hb + mt * 128
                    for ch in range(2):
                        yp = ps2.tile([128, 512], F32, tag="ps", bufs=6,
                                      name=f"yp{hv}_{mt}_{ch}")
                        for p in range(NP):
                            nc.tensor.matmul(
                                yp[:],
                                _mm(att[p][:, mt * 128:(mt + 1) * 128]),
                                _mm(wo_sb[p][:, ch * 512:(ch + 1) * 512]),
                                start=(p == 0), stop=(p == NP - 1))
                        ysb = p2.tile([128, 512], F32, tag="ysb", bufs=3,
                                      name=f"ysb{hv}_{mt}_{ch}")
                        nc.scalar.copy(ysb[:], yp[:])
                        nc.sync.dma_start(
                            out_d[r0:r0 + 128, ch * 512:(ch + 1) * 512],
                            ysb[:])


def _build(has_bias: bool):
    KT = 9 if has_bias else 8
    KC = KT * 128
    cdt = _cdt()

    nc = bacc.Bacc("TRN2", target_bir_lowering=False, debug=False,
                   num_devices=N_CORES)
    xt_d = nc.dram_tensor("xt", [KC, S], cdt, kind="ExternalInput").ap()
    wk_d = nc.dram_tensor("wkt", [KC, C], cdt, kind="ExternalInput").ap()
    wv_d = nc.dram_tensor("wvt", [KC, C], cdt, kind="ExternalInput").ap()
    wq_d = nc.dram_tensor("wqt", [KC, C], cdt, kind="ExternalInput").ap()
    wo_d = nc.dram_tensor("wot", [KC, C], cdt, kind="ExternalInput").ap()
    cs_d = nc.dram_tensor("csel", [2, 128], cdt, kind="ExternalInput").ap()
    out_d = nc.dram_tensor("out", [S, C], F32, kind="ExternalOutput").ap()
    dbg = {}
    if DEBUG_DUMPS:
        dbg["kvcoll"] = nc.dram_tensor(
            "d_kvcoll", [128, NP * PSTR], F32, kind="ExternalOutput").ap()
        dbg["kvagg"] = nc.dram_tensor(
            "d_kvagg", [128, NP * PSTR], F32, kind="ExternalOutput").ap()
        dbg["ktok0"] = nc.dram_tensor(
            "d_ktok0", [128, C], F32, kind="ExternalOutput").ap()
        dbg["vtok0"] = nc.dram_tensor(
            "d_vtok0", [128, C], F32, kind="ExternalOutput").ap()

    with tile.TileContext(nc) as tc:
        for _ in range(REPEAT):
            _emit(nc, tc, KT, xt_d, wk_d, wv_d, wq_d, wo_d, cs_d, out_d, dbg)
    nc.compile()
    return nc


def _prep_host(inputs, KT):
    """Host-side shard + transpose prep. Returns in_maps for the 8 cores."""
    KC = KT * 128
    npdt = mybir.dt.np(_cdt())
    x = np.asarray(inputs["x"], np.float32).reshape(B * T, C)

    def padw(w, b):
        wt = np.ascontiguousarray(np.asarray(w, np.float32).T)  # [C_in, C_out]
        if KC == C:
            return wt.astype(npdt)
        out = np.zeros((KC, C), np.float32)
        out[:C] = wt
        out[C] = np.asarray(b, np.float32)
        return out.astype(npdt)

    wkt = padw(inputs["Wk"], inputs["bk"])
    wvt = padw(inputs["Wv"], inputs["bv"])
    wqt = padw(inputs["Wq"], inputs["bq"])
    wot = padw(inputs["Wo"], np.zeros(C))   # bo applied on host

    csel = np.zeros((2, 128), np.float32)
    csel[0, :64] = 1.0
    csel[1, 64:] = 1.0
    csel = csel.astype(npdt)

    in_maps = []
    for c in range(N_CORES):
        sh = x[c * S:(c + 1) * S]
        xt = np.zeros((KC, S), np.float32)
        xt[:C] = sh.T
        if KC > C:
            xt[C] = 1.0
        in_maps.append({
            "xt": np.ascontiguousarray(xt.astype(npdt)),
            "wkt": wkt, "wvt": wvt, "wqt": wqt, "wot": wot,
            "csel": csel,
        })
    return in_maps


def _get_nc(has_bias):
    key = (COMPUTE, has_bias, DEBUG_DUMPS, REPEAT)
    if key not in _cache:
        _cache[key] = _build(has_bias)
    return _cache[key]


def kernel(**inputs):
    assert np.asarray(inputs["x"]).shape == (B, T, C)
    has_bias = any(
        np.any(np.asarray(inputs[k])) for k in ("bq", "bk", "bv"))
    nc = _get_nc(has_bias)
    in_maps = _prep_host(inputs, 9 if has_bias else 8)
    res = bass_utils.run_bass_kernel_spmd(
        nc, in_maps, core_ids=list(range(N_CORES)))
    y = np.concatenate(
        [res.results[c]["out"] for c in range(N_CORES)], axis=0)
    y = y.reshape(B, T, C).astype(np.float32)
    bo = np.asarray(inputs["bo"], np.float32)
    if np.any(bo):
        y = y + bo
    return y



# revision 2
# speedup vs baseline: 1.2335x; 1.2335x over previous
"""Linear attention (ELU+1 feature map) on 8 TRN2 NeuronCores.

Reference math (per batch b):
    q,k,v = x @ W{q,k,v}.T + b;   q,k -> elu(.)+1
    kv[h,d,e] = sum_t k[t,h,d] v[t,h,e];   ks[h,d] = sum_t k[t,h,d]
    out = ((q kv) / clip(q . ks, 1e-6)) @ Wo.T + bo

Sharding: the 16384 tokens are split into 8 contiguous chunks of 2048; core c
owns batch c//2, T-half c%2. kv/ks are partial sums over the core's tokens,
AllReduce-summed within core pairs {0,1},{2,3},{4,5},{6,7} (one pair = one
batch, adjacent NeuronCores). Everything else is embarrassingly parallel, so
the only communication is a 520 KB pair AllReduce that overlaps the q
projection.

Per-core device program (S=2048 local tokens; a "pair" p = 2 heads = 128
channels; all layouts chosen so no on-device transposes are ever needed):
  phase 1: k,v projections in token-major layout via matmul(lhsT=xT block,
           rhs=W.T stripe). v is written into an interleaved pair layout with
           two ones-columns per pair (single strided 3D-AP copy per PSUM
           chunk), so ONE N=130 matmul per (pair, token-tile) produces both
           the kv outer-product block and the k-sum column, feature-major.
           Partial kv products are DVE-accumulated into SBUF (PSUM
           accumulation groups interleaved within a bank lose their first
           contribution on HW - a sibling group's start=True clears the
           bank's has_written bits).
  phase 2: qT feature-major via matmul(lhsT=Wq.T columns, rhs=xT);
           denominators via a block-diagonal ksum matmul, batched along the
           free dim so one DVE reciprocal serves all pairs; 1/denom is
           broadcast across partitions with a tiny [2,128] selector matmul;
           att = num * (1/denom) stays feature-major and feeds the output
           projection as its stationary operand; y lands token-major in PSUM
           and is copied out through SBUF.

COMPUTE selects the TensorEngine dtype: "f32r" (default) stores f32 bits and
runs the PE in round-trip fp32 mode (full rate at N>=256; ~3.5e-4 rel err),
"bf16" halves the DMA/SBUF footprint (~5.6e-3 rel err), "f32" is the exact
but 4x-slower fallback. Inputs are pre-transposed/sharded on the host; biases
are folded in via an extra ones-row contraction tile only when nonzero (the
bo bias is applied on the host).
"""

import sys
import numpy as np

for _p in ("/opt/trn_rl_repo", "/opt/pypackages"):
    if _p not in sys.path:
        sys.path.append(_p)

import concourse.bacc as bacc
import concourse.mybir as mybir
import concourse.tile as tile
from concourse import bass_utils

F32 = mybir.dt.float32
ACTF = mybir.ActivationFunctionType

N_CORES = 8
B, T, C = 4, 4096, 1024
H, D = 16, 64
S = B * T // N_CORES          # 2048 tokens per core
NP = 8                        # head pairs (128 channels each)
TT = S // 128                 # 16 token tiles per core
HALF = S // 2                 # phase-2 half size (1024)
PSTR = 130                    # kv_aug per-pair stride: 128 kv cols + ksum col
                              # + pad col (f32r matmul needs even N)

COMPUTE = "bf16"              # "f32r" | "bf16" | "f32"
DEBUG_DUMPS = False
REPEAT = 1                    # timing only: emit the body N times in one NEFF

_cache = {}


def _cdt():
    """Storage dtype of matmul-feeding tensors (f32r is f32 bits; the PE
    runs it at full rate when N>=256 and the verifier requires producers
    to declare the f32r dtype end-to-end)."""
    return {"bf16": mybir.dt.bfloat16,
            "f32r": mybir.dt.float32r,
            "f32": F32}[COMPUTE]


def _mm(ap):
    return ap


def _msview(ap):
    """Memset target view: walrus rejects Memset on f32r APs, so write the
    same bits through an f32 view."""
    return ap.bitcast(F32) if COMPUTE == "f32r" else ap


def _emit(nc, tc, KT, xt_d, wk_d, wv_d, wq_d, wo_d, cs_d, out_d, dbg=None):
    dbg = dbg or {}
    cdt = _cdt()
    res_xt = COMPUTE == "bf16"   # 2-byte xT fits SBUF for both phases
    span = HALF if COMPUTE == "bf16" else S // 4
    nchk = span // 512
    Relu, Exp = ACTF.Relu, ACTF.Exp
    WB = 2 * KT  # weight slots: wk+wv in phase 1, reused by wq+wo in phase 2

    with (
        tc.tile_pool(name="wpool", bufs=1) as wpool,
        tc.tile_pool(name="persist", bufs=1) as sb,
        tc.tile_pool(name="dram", bufs=1, space="DRAM") as dram,
    ):
        wk_sb = []
        wv_sb = []
        for ct in range(KT):
            w = wpool.tile([128, C], cdt, tag="w", bufs=WB, name=f"wk{ct}")
            nc.gpsimd.dma_start(w[:], wk_d[ct * 128:(ct + 1) * 128, :])
            wk_sb.append(w)
        for ct in range(KT):
            w = wpool.tile([128, C], cdt, tag="w", bufs=WB, name=f"wv{ct}")
            nc.gpsimd.dma_start(w[:], wv_d[ct * 128:(ct + 1) * 128, :])
            wv_sb.append(w)

        csel = sb.tile([2, 128], cdt, tag="csel", name="csel")
        nc.sync.dma_start(csel[:], cs_d[:])

        kvagg = sb.tile([128, NP * PSTR], F32, tag="kvagg", name="kvagg")

        # ------------- phase 1: k/v projections + kv aggregation -------------
        # NOTE: PSUM accumulation groups interleaved within one bank are
        # broken on HW (a sibling group's start=True clears the bank's
        # has_written bits), so kv partial products are single-shot matmuls
        # accumulated into SBUF by the DVE instead.
        with (
            tc.tile_pool(name="p1sb", bufs=1) as p1,
            tc.tile_pool(name="p1ps", bufs=1, space="PSUM") as ps1,
        ):
            nc.gpsimd.memset(kvagg[:], 0.0)

            # xT stripes: one efficient full-row DMA each (the per-token-tile
            # [128,128] block loads were 512 B/line descriptor-dominated).
            # bf16: allocated from the persistent pool and reused in phase 2.
            xs_pool = sb if res_xt else p1
            xs_sb = []
            for ct in range(KT):
                xst = xs_pool.tile([128, S], cdt, tag="xs", bufs=KT,
                                   name=f"xs{ct}")
                nc.gpsimd.dma_start(xst[:], xt_d[ct * 128:(ct + 1) * 128, :])
                xs_sb.append(xst)

            for tt in range(TT):
                t0 = tt * 128
                xb = [xs_sb[ct][:, t0:t0 + 128] for ct in range(KT)]

                ktok = p1.tile([128, C], cdt, tag="ktok", bufs=3,
                               name=f"ktok{tt}")
                kps, t1s, t2s = [], [], []
                for ch in range(2):
                    kp = ps1.tile([128, 512], F32, tag="ps", bufs=4,
                                  name=f"kp{tt}_{ch}")
                    for ct in range(KT):
                        nc.tensor.matmul(
                            kp[:], _mm(xb[ct]),
                            _mm(wk_sb[ct][:, ch * 512:(ch + 1) * 512]),
                            start=(ct == 0), stop=(ct == KT - 1))
                    kps.append(kp)
                    t1s.append(p1.tile([128, 512], F32, tag="t1", bufs=3,
                                       name=f"t1_{tt}_{ch}"))
                    t2s.append(p1.tile([128, 512], F32, tag="t2", bufs=3,
                                       name=f"t2_{tt}_{ch}"))
                # group by ACT function to avoid per-op table swaps
                for ch in range(2):
                    ks = ktok[:, ch * 512:(ch + 1) * 512]
                    nc.scalar.activation(ks, kps[ch][:], Relu)
                    nc.scalar.activation(t1s[ch][:], kps[ch][:], Relu,
                                         scale=-1.0)
                for ch in range(2):
                    nc.scalar.activation(t2s[ch][:], t1s[ch][:], Exp,
                                         scale=-1.0)
                for ch in range(2):
                    ks = ktok[:, ch * 512:(ch + 1) * 512]
                    nc.vector.tensor_add(ks, ks, t2s[ch][:])

                # v in interleaved pair layout [.. 128 v cols | 2 ones ..]
                # so one N=130 matmul per pair yields kv plus the k-sum.
                # Ones come from a whole-tile memset; v lands via ONE strided
                # 3D-AP copy per psum chunk.
                vaug = p1.tile([128, NP * PSTR], cdt, tag="vaug", bufs=3,
                               name=f"vaug{tt}")
                nc.gpsimd.memset(_msview(vaug[:]), 1.0)
                vau3 = vaug.rearrange("p (g c) -> p g c", c=PSTR)
                for ch in range(2):
                    vp = ps1.tile([128, 512], F32, tag="ps", bufs=4,
                                  name=f"vp{tt}_{ch}")
                    for ct in range(KT):
                        nc.tensor.matmul(
                            vp[:], _mm(xb[ct]),
                            _mm(wv_sb[ct][:, ch * 512:(ch + 1) * 512]),
                            start=(ct == 0), stop=(ct == KT - 1))
                    nc.vector.tensor_copy(
                        vau3[:, ch * 4:(ch + 1) * 4, 0:128],
                        vp[:].rearrange("p (g c) -> p g c", c=128))

                for g in range(3):
                    p0, p1n = 3 * g, min(3 * g + 3, NP)
                    kvt = ps1.tile([128, (p1n - p0) * PSTR], F32, tag="kvt",
                                   bufs=3, name=f"kvt{tt}_{g}",
                                   padded_shape=[128, 3 * PSTR])
                    for p in range(p0, p1n):
                        j = p - p0
                        nc.tensor.matmul(
                            kvt[:, j * PSTR:(j + 1) * PSTR],
                            _mm(ktok[:, p * 128:(p + 1) * 128]),
                            _mm(vaug[:, p * PSTR:(p + 1) * PSTR]),
                            start=True, stop=True)
                    nc.vector.tensor_add(
                        kvagg[:, p0 * PSTR:p1n * PSTR],
                        kvagg[:, p0 * PSTR:p1n * PSTR], kvt[:])

                if tt == 0 and "ktok0" in dbg:
                    kd = p1.tile([128, C], F32, tag="ktd", name="ktd")
                    nc.vector.tensor_copy(kd[:], ktok[:])
                    nc.sync.dma_start(dbg["ktok0"][:], kd[:])
                    vd = p1.tile([128, C], F32, tag="vtd", name="vtd")
                    nc.vector.tensor_copy(vd[:], vtok[:])
                    nc.sync.dma_start(dbg["vtok0"][:], vd[:])


        # ------------- pair AllReduce ----------------------------------------
        bounce_in = dram.tile([128, NP * PSTR], F32, name="bounce_in")
        bounce_out = dram.tile([128, NP * PSTR], F32, name="bounce_out")
        nc.sync.dma_start(bounce_in[:], kvagg[:])
        nc.gpsimd.collective_compute(
            "AllReduce", mybir.AluOpType.add,
            ins=[bounce_in.opt()], outs=[bounce_out.opt()],
            replica_groups=[[2 * i, 2 * i + 1] for i in range(N_CORES // 2)])
        kvcoll = sb.tile([128, NP * PSTR], F32, tag="kvcoll", name="kvcoll")
        nc.sync.dma_start(kvcoll[:], bounce_out[:])
        if "kvcoll" in dbg:
            nc.sync.dma_start(dbg["kvcoll"][:], kvcoll[:])
            nc.sync.dma_start(dbg["kvagg"][:], kvagg[:])

        # phase-2 weights (reuse the phase-1 weight slots)
        wq_sb = []
        wo_sb = []
        for ct in range(KT):
            w = wpool.tile([128, C], cdt, tag="w", bufs=WB, name=f"wq{ct}")
            nc.gpsimd.dma_start(w[:], wq_d[ct * 128:(ct + 1) * 128, :])
            wq_sb.append(w)
        for ct in range(NP):
            w = wpool.tile([128, C], cdt, tag="w", bufs=WB, name=f"wo{ct}")
            nc.gpsimd.dma_start(w[:], wo_d[ct * 128:(ct + 1) * 128, :])
            wo_sb.append(w)

        # block-diagonal kv (cross-head junk zeroed) + ksum column tiles
        kvblk = []
        ksb = []
        for p in range(NP):
            c0 = p * PSTR
            kb = sb.tile([128, 128], cdt, tag="kvblk", bufs=NP,
                         name=f"kvblk{p}")
            nc.gpsimd.memset(_msview(kb[:]), 0.0)
            nc.vector.tensor_copy(kb[0:64, 0:64], kvcoll[0:64, c0:c0 + 64])
            nc.vector.tensor_copy(kb[64:128, 64:128],
                                  kvcoll[64:128, c0 + 64:c0 + 128])
            kvblk.append(kb)
            kt = sb.tile([128, 2], cdt, tag="ksb", bufs=NP, name=f"ksb{p}")
            nc.gpsimd.memset(_msview(kt[:]), 0.0)
            nc.vector.tensor_copy(kt[0:64, 0:1],
                                  kvcoll[0:64, c0 + 128:c0 + 129])
            nc.vector.tensor_copy(kt[64:128, 1:2],
                                  kvcoll[64:128, c0 + 128:c0 + 129])
            ksb.append(kt)

        # ------------- phase 2: q, attention, output projection --------------
        with (
            tc.tile_pool(name="p2sb", bufs=1) as p2,
            tc.tile_pool(name="p2ps", bufs=1, space="PSUM") as ps2,
        ):
            for hv in range(S // span):
                hb = hv * span
                if res_xt:
                    xh = [xs_sb[ct][:, hb:hb + span] for ct in range(KT)]
                else:
                    xh = []
                    for ct in range(KT):
                        xht = p2.tile([128, span], cdt, tag="xh",
                                      bufs=KT + 1, name=f"xh{hv}_{ct}")
                        nc.sync.dma_start(
                            xht[:],
                            xt_d[ct * 128:(ct + 1) * 128, hb:hb + span])
                        xh.append(xht)

                dnb = p2.tile([2, NP * span], F32, tag="dnb", bufs=1,
                              name=f"dnb{hv}")
                qts = []
                for p in range(NP):
                    qt = p2.tile([128, span], cdt, tag="qt", bufs=NP + 1,
                                 name=f"qt{hv}_{p}")
                    qts.append(qt)
                    qps, t1s, t2s = [], [], []
                    for chk in range(nchk):
                        qp = ps2.tile([128, 512], F32, tag="ps", bufs=6,
                                      name=f"qp{hv}_{p}_{chk}")
                        for ct in range(KT):
                            nc.tensor.matmul(
                                qp[:],
                                _mm(wq_sb[ct][:, p * 128:(p + 1) * 128]),
                                _mm(xh[ct][:, chk * 512:(chk + 1) * 512]
                                    if not res_xt else
                                    xs_sb[ct][:, hb + chk * 512:
                                              hb + (chk + 1) * 512]),
                                start=(ct == 0), stop=(ct == KT - 1))
                        qps.append(qp)
                        t1s.append(p2.tile([128, 512], F32, tag="qt1",
                                           bufs=3, name=f"qt1_{hv}_{p}_{chk}"))
                        t2s.append(p2.tile([128, 512], F32, tag="qt2",
                                           bufs=3, name=f"qt2_{hv}_{p}_{chk}"))
                    for chk in range(nchk):
                        qs = qt[:, chk * 512:(chk + 1) * 512]
                        nc.scalar.activation(qs, qps[chk][:], Relu)
                        nc.scalar.activation(t1s[chk][:], qps[chk][:], Relu,
                                             scale=-1.0)
                    for chk in range(nchk):
                        nc.scalar.activation(t2s[chk][:], t1s[chk][:], Exp,
                                             scale=-1.0)
                    for chk in range(nchk):
                        qs = qt[:, chk * 512:(chk + 1) * 512]
                        nc.vector.tensor_add(qs, qs, t2s[chk][:])

                    for chk in range(nchk):
                        dn = ps2.tile([2, 512], F32, tag="dn", bufs=2,
                                      name=f"dn{hv}_{p}_{chk}")
                        nc.tensor.matmul(
                            dn[:], _mm(ksb[p][:]),
                            _mm(qt[:, chk * 512:(chk + 1) * 512]),
                            start=True, stop=True)
                        nc.vector.tensor_scalar_max(
                            dnb[:, p * span + chk * 512:
                                p * span + (chk + 1) * 512], dn[:], 1e-6)

                recb = p2.tile([2, NP * span], cdt, tag="recb", bufs=1,
                               name=f"recb{hv}")
                with nc.allow_low_precision(reason="recip of clipped denom"):
                    nc.vector.reciprocal(recb[:], dnb[:])

                att = []
                for p in range(NP):
                    qt = qts[p]
                    at = p2.tile([128, span], cdt, tag="att", bufs=NP + 1,
                                 name=f"att{hv}_{p}")
                    for chk in range(nchk):
                        nm = ps2.tile([128, 512], F32, tag="ps", bufs=6,
                                      name=f"nm{hv}_{p}_{chk}")
                        nc.tensor.matmul(
                            nm[:], _mm(kvblk[p][:]),
                            _mm(qt[:, chk * 512:(chk + 1) * 512]),
                            start=True, stop=True)
                        rp = ps2.tile([128, 512], F32, tag="ps", bufs=6,
                                      name=f"rp{hv}_{p}_{chk}")
                        nc.tensor.matmul(
                            rp[:], _mm(csel[:]),
                            _mm(recb[:, p * span + chk * 512:
                                     p * span + (chk + 1) * 512]),
                            start=True, stop=True)
                        ats = at[:, chk * 512:(chk + 1) * 512]
                        nc.scalar.copy(ats, nm[:])
                        nc.vector.tensor_mul(ats, ats, rp[:])
                    att.append(at)

                for mt in range(span // 128):
                    r0 = hb + mt * 128
                    for ch in range(2):
                        yp = ps2.tile([128, 512], F32, tag="ps", bufs=6,
                                      name=f"yp{hv}_{mt}_{ch}")
                        for p in range(NP):
                            nc.tensor.matmul(
                                yp[:],
                                _mm(att[p][:, mt * 128:(mt + 1) * 128]),
                                _mm(wo_sb[p][:, ch * 512:(ch + 1) * 512]),
                                start=(p == 0), stop=(p == NP - 1))
                        ysb = p2.tile([128, 512], F32, tag="ysb", bufs=3,
                                      name=f"ysb{hv}_{mt}_{ch}")
                        nc.scalar.copy(ysb[:], yp[:])
                        nc.sync.dma_start(
                            out_d[r0:r0 + 128, ch * 512:(ch + 1) * 512],
                            ysb[:])


def _build(has_bias: bool):
    KT = 9 if has_bias else 8
    KC = KT * 128
    cdt = _cdt()

    nc = bacc.Bacc("TRN2", target_bir_lowering=False, debug=False,
                   num_devices=N_CORES)
    xt_d = nc.dram_tensor("xt", [KC, S], cdt, kind="ExternalInput").ap()
    wk_d = nc.dram_tensor("wkt", [KC, C], cdt, kind="ExternalInput").ap()
    wv_d = nc.dram_tensor("wvt", [KC, C], cdt, kind="ExternalInput").ap()
    wq_d = nc.dram_tensor("wqt", [KC, C], cdt, kind="ExternalInput").ap()
    wo_d = nc.dram_tensor("wot", [KC, C], cdt, kind="ExternalInput").ap()
    cs_d = nc.dram_tensor("csel", [2, 128], cdt, kind="ExternalInput").ap()
    out_d = nc.dram_tensor("out", [S, C], F32, kind="ExternalOutput").ap()
    dbg = {}
    if DEBUG_DUMPS:
        dbg["kvcoll"] = nc.dram_tensor(
            "d_kvcoll", [128, NP * PSTR], F32, kind="ExternalOutput").ap()
        dbg["kvagg"] = nc.dram_tensor(
            "d_kvagg", [128, NP * PSTR], F32, kind="ExternalOutput").ap()
        dbg["ktok0"] = nc.dram_tensor(
            "d_ktok0", [128, C], F32, kind="ExternalOutput").ap()
        dbg["vtok0"] = nc.dram_tensor(
            "d_vtok0", [128, C], F32, kind="ExternalOutput").ap()

    with tile.TileContext(nc) as tc:
        for _ in range(REPEAT):
            _emit(nc, tc, KT, xt_d, wk_d, wv_d, wq_d, wo_d, cs_d, out_d, dbg)
    nc.compile()
    return nc


def _prep_host(inputs, KT):
    """Host-side shard + transpose prep. Returns in_maps for the 8 cores."""
    KC = KT * 128
    npdt = mybir.dt.np(_cdt())
    x = np.asarray(inputs["x"], np.float32).reshape(B * T, C)

    def padw(w, b):
        wt = np.ascontiguousarray(np.asarray(w, np.float32).T)  # [C_in, C_out]
        if KC == C:
            return wt.astype(npdt)
        out = np.zeros((KC, C), np.float32)
        out[:C] = wt
        out[C] = np.asarray(b, np.float32)
        return out.astype(npdt)

    wkt = padw(inputs["Wk"], inputs["bk"])
    wvt = padw(inputs["Wv"], inputs["bv"])
    wqt = padw(inputs["Wq"], inputs["bq"])
    wot = padw(inputs["Wo"], np.zeros(C))   # bo applied on host

    csel = np.zeros((2, 128), np.float32)
    csel[0, :64] = 1.0
    csel[1, 64:] = 1.0
    csel = csel.astype(npdt)

    in_maps = []
    for c in range(N_CORES):
        sh = x[c * S:(c + 1) * S]
        xt = np.zeros((KC, S), np.float32)
        xt[:C] = sh.T
        if KC > C:
            xt[C] = 1.0
        in_maps.append({
            "xt": np.ascontiguousarray(xt.astype(npdt)),
            "wkt": wkt, "wvt": wvt, "wqt": wqt, "wot": wot,
            "csel": csel,
        })
    return in_maps


def _get_nc(has_bias):
    key = (COMPUTE, has_bias, DEBUG_DUMPS, REPEAT)
    if key not in _cache:
        _cache[key] = _build(has_bias)
    return _cache[key]


def kernel(**inputs):
    assert np.asarray(inputs["x"]).shape == (B, T, C)
    has_bias = any(
        np.any(np.asarray(inputs[k])) for k in ("bq", "bk", "bv"))
    nc = _get_nc(has_bias)
    in_maps = _prep_host(inputs, 9 if has_bias else 8)
    res = bass_utils.run_bass_kernel_spmd(
        nc, in_maps, core_ids=list(range(N_CORES)))
    y = np.concatenate(
        [res.results[c]["out"] for c in range(N_CORES)], axis=0)
    y = y.reshape(B, T, C).astype(np.float32)
    bo = np.asarray(inputs["bo"], np.float32)
    if np.any(bo):
        y = y + bo
    return y



# revision 19
# speedup vs baseline: 1.8952x; 1.5365x over previous
"""Linear attention (ELU+1 feature map) on 8 TRN2 NeuronCores.

Reference math (per batch b):
    q,k,v = x @ W{q,k,v}.T + b;   q,k -> elu(.)+1
    kv[h,d,e] = sum_t k[t,h,d] v[t,h,e];   ks[h,d] = sum_t k[t,h,d]
    out = ((q kv) / clip(q . ks, 1e-6)) @ Wo.T + bo

Sharding: the 16384 tokens are split into 8 contiguous chunks of 2048; core c
owns batch c//2, T-half c%2. kv/ks are partial sums over the core's tokens,
AllReduce-summed within core pairs {0,1},{2,3},{4,5},{6,7}. Everything else is
embarrassingly parallel; the 520 KB pair AllReduce overlaps the q projection.

Fast path (zero projection biases, the common case): the q/k projections run
as fp8(e4m3) DoubleRow matmuls (x1 = fp8(x*SX), W1 = fp8(W.T*SW); the fp8
quantization error of q and k largely cancels between the attention numerator
q.kv and denominator q.ks since both are linear in q and in k -- measured
~9.4e-3 rel err end to end). The v projection needs real precision, so it uses
a 3-term error-compensated fp8 product x1@Wv1 + x1@Wv2 + x2@Wv1 (residuals
quantized at the SAME power-2 scale so all terms share one PSUM accumulation
chain; the dropped x2@Wv2 term is O(fp8_eps^2) ~ bf16 level). DoubleRow packs
two 128-deep contraction tiles per instruction, so q/k cost 1/4 and v 3/4 of
a bf16 projection on the PE. Everything downstream (kv outer products with the
ones-column ksum trick, num/denominator/reciprocal-broadcast matmuls, output
projection) stays bf16.

Engine balance: the elu+1 chain relu(x) + exp(-relu(-x)) is split so no
single engine throttles the PE: relu/exp on ACT, the -relu(-x) on ACT in
phase 1 but DVE in phase 2, the final add on the (otherwise idle) Pool
engine, v unscale-copies and kv aggregation on DVE. kv outer products are
emitted one token-tile behind the projections so the ACT->Pool chain latency
never stalls the PE. Input DMAs are token-chunked and issued on the cheap
gpsimd queue in consumption order so the first matmul starts ~5 us in.

Fallback path (nonzero biases): the original bf16 two-phase kernel with a
ones-row bias fold.
"""

import sys
import numpy as np

for _p in ("/opt/trn_rl_repo", "/opt/pypackages"):
    if _p not in sys.path:
        sys.path.append(_p)

import concourse.bacc as bacc
import concourse.mybir as mybir
import concourse.tile as tile
from concourse import bass_utils

F32 = mybir.dt.float32
BF16 = mybir.dt.bfloat16
FP8 = mybir.dt.float8e4
ACTF = mybir.ActivationFunctionType
DROW = mybir.MatmulPerfMode.DoubleRow

N_CORES = 8
B, T, C = 4, 4096, 1024
H, D = 16, 64
S = B * T // N_CORES          # 2048 tokens per core
NP = 8                        # head pairs (128 channels each)
TT = S // 128                 # 16 token tiles per core
PSTR = 130                    # kv_aug per-pair stride: 128 kv cols + ksum col
                              # + pad col
NJ = 4                        # fp8 stripe pairs (2x128 contraction each)

SX = 2.0                      # fp8 x scale (power of 2)
SW = 512.0                    # fp8 weight scale (power of 2)
US = 1.0 / (SX * SW)          # PSUM unscale

_cache = {}


# ---------------------------------------------------------------------------
# fast path: fp8 DoubleRow projections, zero biases
# ---------------------------------------------------------------------------

def _emit_fp8(nc, tc, x1_d, x2_d, wk1_d, wq1_d, wv1_d, wv2_d, wo_d, cs_d,
              out_d):
    Relu, Exp, Copy = ACTF.Relu, ACTF.Exp, ACTF.Copy
    XCH = 1024                # x DMA token chunk

    with (
        tc.tile_pool(name="sb", bufs=1) as sb,
        tc.tile_pool(name="ps", bufs=1, space="PSUM") as ps,
        tc.tile_pool(name="dram", bufs=1, space="DRAM") as dram,
    ):
        # ---- input DMAs in consumption order on the sync queue: HWDGE
        # descriptor-gen (~0.6 us, off-engine) instead of gpsimd SWDGE
        # (~1.8 us on the Pool engine, which runs the elu adds) ----
        def wtiles(src, tag, eng=None):
            ts = []
            for j in range(NJ):
                w = sb.tile([128, 2 * C], FP8, tag=tag, bufs=NJ,
                            name=f"{tag}{j}")
                (eng or nc.sync).dma_start(w[:], src[j * 128:(j + 1) * 128, :])
                ts.append(w)
            return ts

        wk1 = wtiles(wk1_d, "wk1", eng=nc.gpsimd)

        X1 = [sb.tile([128, 2 * S], FP8, tag="x1", bufs=NJ, name=f"x1_{j}")
              for j in range(NJ)]
        X2 = [sb.tile([128, 2 * S], FP8, tag="x2", bufs=NJ, name=f"x2_{j}")
              for j in range(NJ)]

        def load_x(dst, src_d, c):
            for j in range(NJ):
                for i in range(2):
                    o = i * S + c * XCH
                    nc.sync.dma_start(
                        dst[j][:, o:o + XCH],
                        src_d[j * 128:(j + 1) * 128, o:o + XCH])

        load_x(X1, x1_d, 0)
        wv1 = wtiles(wv1_d, "wv1")
        wv2 = wtiles(wv2_d, "wv2")
        load_x(X2, x2_d, 0)
        load_x(X1, x1_d, 1)
        load_x(X2, x2_d, 1)
        wq1 = wtiles(wq1_d, "wq1")
        wo = []
        for p in range(NP):
            w = sb.tile([128, C], BF16, tag="wo", bufs=NP, name=f"wo{p}")
            nc.sync.dma_start(w[:], wo_d[p * 128:(p + 1) * 128, :])
            wo.append(w)
        csel = sb.tile([2, 128], BF16, tag="csel", name="csel")
        nc.sync.dma_start(csel[:], cs_d[:])

        x1v = [t.rearrange("p (i s) -> p i s", i=2) for t in X1]
        x2v = [t.rearrange("p (i s) -> p i s", i=2) for t in X2]
        wk1v = [t.rearrange("p (i s) -> p i s", i=2) for t in wk1]
        wq1v = [t.rearrange("p (i s) -> p i s", i=2) for t in wq1]
        wv1v = [t.rearrange("p (i s) -> p i s", i=2) for t in wv1]
        wv2v = [t.rearrange("p (i s) -> p i s", i=2) for t in wv2]

        kvagg = sb.tile([128, NP * PSTR], F32, tag="kvagg", name="kvagg")
        nc.gpsimd.memset(kvagg[:], 0.0)

        # ------------- phase 1: k/v projections + kv aggregation -----------
        # kv matmuls run one token-tile behind the projections so the
        # ACT->Pool elu chain latency never stalls the PE.
        pend = None

        def flush_kv(ktok, vaug, tt):
            for g in range(3):
                p0, p1n = 3 * g, min(3 * g + 3, NP)
                kvt = ps.tile([128, (p1n - p0) * PSTR], F32, tag="qps",
                              bufs=2, name=f"kvt{tt}_{g}",
                              padded_shape=[128, 512])
                for p in range(p0, p1n):
                    j = p - p0
                    nc.tensor.matmul(
                        kvt[:, j * PSTR:(j + 1) * PSTR],
                        ktok[:, p * 128:(p + 1) * 128],
                        vaug[:, p * PSTR:(p + 1) * PSTR],
                        start=True, stop=True)
                nc.vector.tensor_add(
                    kvagg[:, p0 * PSTR:p1n * PSTR],
                    kvagg[:, p0 * PSTR:p1n * PSTR], kvt[:])

        for tt in range(TT):
            t0 = tt * 128
            ktok = sb.tile([128, C], BF16, tag="ktok", bufs=4,
                           name=f"ktok{tt}")
            t1 = sb.tile([128, C], BF16, tag="t1", bufs=4, name=f"t1_{tt}")
            t2 = sb.tile([128, C], BF16, tag="t2", bufs=4, name=f"t2_{tt}")
            for ch in range(2):
                kp = ps.tile([128, 512], F32, tag="ps", bufs=6,
                             name=f"kp{tt}_{ch}")
                for j in range(NJ):
                    nc.tensor.matmul(
                        kp[:],
                        x1v[j][:, :, t0:t0 + 128],
                        wk1v[j][:, :, ch * 512:(ch + 1) * 512],
                        start=(j == 0), stop=(j == NJ - 1), perf_mode=DROW)
                cs = slice(ch * 512, (ch + 1) * 512)
                nc.scalar.activation(ktok[:, cs], kp[:], Relu, scale=US)
                nc.vector.tensor_scalar(t1[:, cs], kp[:], -US, 0.0,
                                        mybir.AluOpType.mult,
                                        mybir.AluOpType.max)
                nc.scalar.activation(t2[:, cs], t1[:, cs], Exp, scale=-1.0)
                nc.gpsimd.tensor_add(ktok[:, cs], ktok[:, cs], t2[:, cs])

            vaug = sb.tile([128, NP * PSTR], BF16, tag="vaug", bufs=4,
                           name=f"vaug{tt}")
            nc.gpsimd.memset(vaug[:], 1.0)
            vau3 = vaug.rearrange("p (g c) -> p g c", c=PSTR)
            for ch in range(2):
                vp = ps.tile([128, 512], F32, tag="ps", bufs=6,
                             name=f"vp{tt}_{ch}")
                n = 0
                for xv, wv in ((x1v, wv1v), (x1v, wv2v), (x2v, wv1v)):
                    for j in range(NJ):
                        nc.tensor.matmul(
                            vp[:],
                            xv[j][:, :, t0:t0 + 128],
                            wv[j][:, :, ch * 512:(ch + 1) * 512],
                            start=(n == 0), stop=(n == 3 * NJ - 1),
                            perf_mode=DROW)
                        n += 1
                dst = vau3[:, ch * 4:(ch + 1) * 4, 0:128]
                src = vp[:].rearrange("p (g c) -> p g c", c=128)
                nc.scalar.activation(dst, src, Copy, scale=US)

            if pend is not None:
                flush_kv(*pend)
            pend = (ktok, vaug, tt)
        flush_kv(*pend)

        # ------------- pair AllReduce --------------------------------------
        bounce_in = dram.tile([128, NP * PSTR], F32, name="bounce_in")
        bounce_out = dram.tile([128, NP * PSTR], F32, name="bounce_out")
        nc.sync.dma_start(bounce_in[:], kvagg[:])
        nc.gpsimd.collective_compute(
            "AllReduce", mybir.AluOpType.add,
            ins=[bounce_in.opt()], outs=[bounce_out.opt()],
            replica_groups=[[2 * i, 2 * i + 1] for i in range(N_CORES // 2)])
        kvcoll = sb.tile([128, NP * PSTR], F32, tag="kvcoll", name="kvcoll")
        nc.sync.dma_start(kvcoll[:], bounce_out[:])

        # block-diagonal kv (cross-head junk zeroed) + ksum column tiles
        kvblk = []
        ksb = []
        for p in range(NP):
            c0 = p * PSTR
            kt = sb.tile([128, 2], BF16, tag="ksb", bufs=NP, name=f"ksb{p}")
            nc.gpsimd.memset(kt[:], 0.0)
            nc.vector.tensor_copy(kt[0:64, 0:1],
                                  kvcoll[0:64, c0 + 128:c0 + 129])
            nc.vector.tensor_copy(kt[64:128, 1:2],
                                  kvcoll[64:128, c0 + 128:c0 + 129])
            ksb.append(kt)
        for p in range(NP):
            c0 = p * PSTR
            kb = sb.tile([128, 128], BF16, tag="kvblk", bufs=NP,
                         name=f"kvblk{p}")
            nc.gpsimd.memset(kb[:], 0.0)
            nc.scalar.activation(kb[0:64, 0:64], kvcoll[0:64, c0:c0 + 64],
                                 ACTF.Copy)
            nc.scalar.activation(kb[64:128, 64:128],
                                 kvcoll[64:128, c0 + 64:c0 + 128],
                                 ACTF.Copy)
            kvblk.append(kb)

        # ------------- phase 2: q, attention, output projection ------------
        # Section order: q(hv0) | norm(hv0) | q(hv1) | out(hv0) | norm(hv1)
        # | out(hv1).  The engines run their queues in order, so hv1's
        # ACT-heavy q chains must NOT sit between hv0's norm and hv0's
        # ACT-light output projection; interleaving this way keeps every
        # engine's queue aligned with the PE's needs.  q(hv0) also covers
        # the AllReduce + kvblk/ksb prep latency.
        HSPAN = 1024
        NHV = S // HSPAN

        def q_chains(hv):
            hb = hv * HSPAN
            qts = []
            for p in range(NP):
                qt = sb.tile([128, HSPAN], BF16, tag="qt", bufs=NP + 2,
                             name=f"qt{hv}_{p}")
                t1 = sb.tile([128, HSPAN], BF16, tag="t1", bufs=4,
                             name=f"qt1_{hv}_{p}")
                t2 = sb.tile([128, HSPAN], BF16, tag="t2", bufs=4,
                             name=f"qt2_{hv}_{p}")
                for chk in range(2):
                    c0 = hb + chk * 512
                    qp = ps.tile([128, 512], F32, tag="qps", bufs=2,
                                 name=f"qp{hv}_{p}_{chk}")
                    for j in range(NJ):
                        nc.tensor.matmul(
                            qp[:],
                            wq1v[j][:, :, p * 128:(p + 1) * 128],
                            x1v[j][:, :, c0:c0 + 512],
                            start=(j == 0), stop=(j == NJ - 1),
                            perf_mode=DROW)
                    cs = slice(chk * 512, (chk + 1) * 512)
                    nc.scalar.activation(qt[:, cs], qp[:], Relu, scale=US)
                    nc.vector.tensor_scalar(t1[:, cs], qp[:], -US, 0.0,
                                            mybir.AluOpType.mult,
                                            mybir.AluOpType.max)
                    nc.scalar.activation(t2[:, cs], t1[:, cs], Exp,
                                         scale=-1.0)
                    nc.gpsimd.tensor_add(qt[:, cs], qt[:, cs], t2[:, cs])
                qts.append(qt)
            return qts

        def norm_section(hv, qts):
            # The PE runs strictly in order, so a reciprocal-blocked rp
            # matmul would stall every later matmul behind it.  Wave order:
            # all denominators (reciprocals compute on DVE meanwhile), then
            # all numerators, then all rp broadcasts — by the time the first
            # rp issues, its reciprocal is long done.
            att = []
            recbs = []
            for p in range(NP):
                qt = qts[p]
                recb = sb.tile([2, HSPAN], BF16, tag="recb", bufs=NP + 1,
                               name=f"recb{hv}_{p}")
                for chk in range(2):
                    dn = ps.tile([2, 512], F32, tag="ps", bufs=6,
                                 name=f"dn{hv}_{p}_{chk}")
                    nc.tensor.matmul(
                        dn[:], ksb[p][:],
                        qt[:, chk * 512:(chk + 1) * 512],
                        start=True, stop=True)
                    # denom = q.ks with q,ks > 0 elementwise (elu+1), so the
                    # reference clip(1e-6) is unreachable; invert directly.
                    with nc.allow_low_precision(reason="recip of pos denom"):
                        nc.vector.reciprocal(
                            recb[:, chk * 512:(chk + 1) * 512], dn[:])
                recbs.append(recb)

            for p in range(NP):
                at = sb.tile([128, HSPAN], BF16, tag="att", bufs=NP + 2,
                             name=f"att{hv}_{p}")
                for chk in range(2):
                    cs = slice(chk * 512, (chk + 1) * 512)
                    nm = ps.tile([128, 512], F32, tag="ps", bufs=6,
                                 name=f"nm{hv}_{p}_{chk}")
                    nc.tensor.matmul(nm[:], kvblk[p][:], qts[p][:, cs],
                                     start=True, stop=True)
                    # DVE can read only one PSUM operand: stage nm via ACT
                    nc.scalar.activation(at[:, cs], nm[:], Copy)
                att.append(at)

            for p in range(NP):
                for chk in range(2):
                    cs = slice(chk * 512, (chk + 1) * 512)
                    rp = ps.tile([128, 512], F32, tag="ps", bufs=6,
                                 name=f"rp{hv}_{p}_{chk}")
                    nc.tensor.matmul(rp[:], csel[:], recbs[p][:, cs],
                                     start=True, stop=True)
                    nc.vector.tensor_mul(att[p][:, cs], att[p][:, cs],
                                         rp[:])
            return att

        def out_section(hv, att):
            hb = hv * HSPAN
            for mt in range(HSPAN // 128):
                r0 = hb + mt * 128
                ysb = sb.tile([128, 1024], F32, tag="ysb", bufs=3,
                              name=f"ysb{hv}_{mt}")
                for ch in range(2):
                    yp = ps.tile([128, 512], F32, tag="ps", bufs=6,
                                 name=f"yp{hv}_{mt}_{ch}")
                    for p in range(NP):
                        nc.tensor.matmul(
                            yp[:],
                            att[p][:, mt * 128:(mt + 1) * 128],
                            wo[p][:, ch * 512:(ch + 1) * 512],
                            start=(p == 0), stop=(p == NP - 1))
                    # split ysb staging across ACT and DVE
                    if ch == 0:
                        nc.scalar.activation(
                            ysb[:, ch * 512:(ch + 1) * 512], yp[:], Copy)
                    else:
                        nc.vector.tensor_copy(
                            ysb[:, ch * 512:(ch + 1) * 512], yp[:])
                # alternate out DMAs between the gpsimd queue (SWDGE on the
                # tail-idle Pool engine) and the sync queue (HWDGE, idle
                # after the input loads) to halve per-queue serialization.
                eng = nc.gpsimd if mt % 2 == 0 else nc.sync
                eng.dma_start(out_d[r0:r0 + 128, :], ysb[:])

        q0 = q_chains(0)
        att0 = norm_section(0, q0)
        q1 = q_chains(1)
        out_section(0, att0)
        att1 = norm_section(1, q1)
        out_section(1, att1)


def _build_fp8():
    nc = bacc.Bacc("TRN2", target_bir_lowering=False, debug=False,
                   num_devices=N_CORES)
    x1_d = nc.dram_tensor("x1", [NJ * 128, 2 * S], FP8,
                          kind="ExternalInput").ap()
    x2_d = nc.dram_tensor("x2", [NJ * 128, 2 * S], FP8,
                          kind="ExternalInput").ap()
    wk1_d = nc.dram_tensor("wk1", [NJ * 128, 2 * C], FP8,
                           kind="ExternalInput").ap()
    wq1_d = nc.dram_tensor("wq1", [NJ * 128, 2 * C], FP8,
                           kind="ExternalInput").ap()
    wv1_d = nc.dram_tensor("wv1", [NJ * 128, 2 * C], FP8,
                           kind="ExternalInput").ap()
    wv2_d = nc.dram_tensor("wv2", [NJ * 128, 2 * C], FP8,
                           kind="ExternalInput").ap()
    wo_d = nc.dram_tensor("wo", [C, C], BF16, kind="ExternalInput").ap()
    cs_d = nc.dram_tensor("csel", [2, 128], BF16, kind="ExternalInput").ap()
    out_d = nc.dram_tensor("out", [S, C], F32, kind="ExternalOutput").ap()

    with tile.TileContext(nc) as tc:
        _emit_fp8(nc, tc, x1_d, x2_d, wk1_d, wq1_d, wv1_d, wv2_d, wo_d,
                  cs_d, out_d)
    nc.compile()
    return nc


def _pack_pairs(a):
    """[C, N] -> stripe-pair packed [NJ*128, 2*N] (j, r, i, t)."""
    Cin, N = a.shape
    return np.ascontiguousarray(
        a.reshape(NJ, 2, 128, N).transpose(0, 2, 1, 3).reshape(NJ * 128,
                                                               2 * N))


def _prep_host_fp8(inputs):
    f8 = mybir.dt.np(FP8)
    bf = mybir.dt.np(BF16)
    x = np.asarray(inputs["x"], np.float32).reshape(B * T, C)

    def q8(a):
        return (a * SW).astype(f8)

    wk1 = _pack_pairs(q8(np.ascontiguousarray(
        np.asarray(inputs["Wk"], np.float32).T)))
    wq1 = _pack_pairs(q8(np.ascontiguousarray(
        np.asarray(inputs["Wq"], np.float32).T)))
    wvt = np.ascontiguousarray(np.asarray(inputs["Wv"], np.float32).T)
    wv1q = q8(wvt)
    wv1 = _pack_pairs(wv1q)
    wv2 = _pack_pairs(q8(wvt - wv1q.astype(np.float32) / SW))
    wot = np.ascontiguousarray(
        np.asarray(inputs["Wo"], np.float32).T).astype(bf)

    csel = np.zeros((2, 128), np.float32)
    csel[0, :64] = 1.0
    csel[1, 64:] = 1.0
    csel = csel.astype(bf)

    in_maps = []
    for c in range(N_CORES):
        sh = np.ascontiguousarray(x[c * S:(c + 1) * S].T)  # [C, S]
        x1q = (sh * SX).astype(f8)
        x2q = ((sh - x1q.astype(np.float32) / SX) * SX).astype(f8)
        in_maps.append({
            "x1": _pack_pairs(x1q),
            "x2": _pack_pairs(x2q),
            "wk1": wk1, "wq1": wq1, "wv1": wv1, "wv2": wv2,
            "wo": wot, "csel": csel,
        })
    return in_maps


# ---------------------------------------------------------------------------
# fallback path (nonzero biases): original bf16 two-phase kernel
# ---------------------------------------------------------------------------

def _emit_bias(nc, tc, KT, xt_d, wk_d, wv_d, wq_d, wo_d, cs_d, out_d):
    S4 = S // 2
    nchk = S4 // 512
    Relu, Exp = ACTF.Relu, ACTF.Exp
    WB = 2 * KT

    with (
        tc.tile_pool(name="wpool", bufs=1) as wpool,
        tc.tile_pool(name="persist", bufs=1) as sb,
        tc.tile_pool(name="dram", bufs=1, space="DRAM") as dram,
    ):
        wk_sb = []
        wv_sb = []
        for ct in range(KT):
            w = wpool.tile([128, C], BF16, tag="w", bufs=WB, name=f"wk{ct}")
            nc.gpsimd.dma_start(w[:], wk_d[ct * 128:(ct + 1) * 128, :])
            wk_sb.append(w)
        for ct in range(KT):
            w = wpool.tile([128, C], BF16, tag="w", bufs=WB, name=f"wv{ct}")
            nc.gpsimd.dma_start(w[:], wv_d[ct * 128:(ct + 1) * 128, :])
            wv_sb.append(w)

        csel = sb.tile([2, 128], BF16, tag="csel", name="csel")
        nc.sync.dma_start(csel[:], cs_d[:])

        kvagg = sb.tile([128, NP * PSTR], F32, tag="kvagg", name="kvagg")

        with (
            tc.tile_pool(name="p1sb", bufs=1) as p1,
            tc.tile_pool(name="p1ps", bufs=1, space="PSUM") as ps1,
        ):
            nc.gpsimd.memset(kvagg[:], 0.0)

            xs_sb = []
            for ct in range(KT):
                xst = sb.tile([128, S], BF16, tag="xs", bufs=KT,
                              name=f"xs{ct}")
                nc.gpsimd.dma_start(xst[:], xt_d[ct * 128:(ct + 1) * 128, :])
                xs_sb.append(xst)

            for tt in range(TT):
                t0 = tt * 128
                xb = [xs_sb[ct][:, t0:t0 + 128] for ct in range(KT)]

                ktok = p1.tile([128, C], BF16, tag="ktok", bufs=3,
                               name=f"ktok{tt}")
                kps, t1s, t2s = [], [], []
                for ch in range(2):
                    kp = ps1.tile([128, 512], F32, tag="ps", bufs=4,
                                  name=f"kp{tt}_{ch}")
                    for ct in range(KT):
                        nc.tensor.matmul(
                            kp[:], xb[ct],
                            wk_sb[ct][:, ch * 512:(ch + 1) * 512],
                            start=(ct == 0), stop=(ct == KT - 1))
                    kps.append(kp)
                    t1s.append(p1.tile([128, 512], F32, tag="t1", bufs=3,
                                       name=f"t1_{tt}_{ch}"))
                    t2s.append(p1.tile([128, 512], F32, tag="t2", bufs=3,
                                       name=f"t2_{tt}_{ch}"))
                for ch in range(2):
                    ks = ktok[:, ch * 512:(ch + 1) * 512]
                    nc.scalar.activation(ks, kps[ch][:], Relu)
                    nc.scalar.activation(t1s[ch][:], kps[ch][:], Relu,
                                         scale=-1.0)
                for ch in range(2):
                    nc.scalar.activation(t2s[ch][:], t1s[ch][:], Exp,
                                         scale=-1.0)
                for ch in range(2):
                    ks = ktok[:, ch * 512:(ch + 1) * 512]
                    nc.vector.tensor_add(ks, ks, t2s[ch][:])

                vaug = p1.tile([128, NP * PSTR], BF16, tag="vaug", bufs=3,
                               name=f"vaug{tt}")
                nc.gpsimd.memset(vaug[:], 1.0)
                vau3 = vaug.rearrange("p (g c) -> p g c", c=PSTR)
                for ch in range(2):
                    vp = ps1.tile([128, 512], F32, tag="ps", bufs=4,
                                  name=f"vp{tt}_{ch}")
                    for ct in range(KT):
                        nc.tensor.matmul(
                            vp[:], xb[ct],
                            wv_sb[ct][:, ch * 512:(ch + 1) * 512],
                            start=(ct == 0), stop=(ct == KT - 1))
                    nc.vector.tensor_copy(
                        vau3[:, ch * 4:(ch + 1) * 4, 0:128],
                        vp[:].rearrange("p (g c) -> p g c", c=128))

                for g in range(3):
                    p0, p1n = 3 * g, min(3 * g + 3, NP)
                    kvt = ps1.tile([128, (p1n - p0) * PSTR], F32, tag="kvt",
                                   bufs=3, name=f"kvt{tt}_{g}",
                                   padded_shape=[128, 3 * PSTR])
                    for p in range(p0, p1n):
                        j = p - p0
                        nc.tensor.matmul(
                            kvt[:, j * PSTR:(j + 1) * PSTR],
                            ktok[:, p * 128:(p + 1) * 128],
                            vaug[:, p * PSTR:(p + 1) * PSTR],
                            start=True, stop=True)
                    nc.vector.tensor_add(
                        kvagg[:, p0 * PSTR:p1n * PSTR],
                        kvagg[:, p0 * PSTR:p1n * PSTR], kvt[:])

        bounce_in = dram.tile([128, NP * PSTR], F32, name="bounce_in")
        bounce_out = dram.tile([128, NP * PSTR], F32, name="bounce_out")
        nc.sync.dma_start(bounce_in[:], kvagg[:])
        nc.gpsimd.collective_compute(
            "AllReduce", mybir.AluOpType.add,
            ins=[bounce_in.opt()], outs=[bounce_out.opt()],
            replica_groups=[[2 * i, 2 * i + 1] for i in range(N_CORES // 2)])
        kvcoll = sb.tile([128, NP * PSTR], F32, tag="kvcoll", name="kvcoll")
        nc.sync.dma_start(kvcoll[:], bounce_out[:])

        wq_sb = []
        wo_sb = []
        for ct in range(KT):
            w = wpool.tile([128, C], BF16, tag="w", bufs=WB, name=f"wq{ct}")
            nc.gpsimd.dma_start(w[:], wq_d[ct * 128:(ct + 1) * 128, :])
            wq_sb.append(w)
        for ct in range(NP):
            w = wpool.tile([128, C], BF16, tag="w", bufs=WB, name=f"wo{ct}")
            nc.gpsimd.dma_start(w[:], wo_d[ct * 128:(ct + 1) * 128, :])
            wo_sb.append(w)

        kvblk = []
        ksb = []
        for p in range(NP):
            c0 = p * PSTR
            kb = sb.tile([128, 128], BF16, tag="kvblk", bufs=NP,
                         name=f"kvblk{p}")
            nc.gpsimd.memset(kb[:], 0.0)
            nc.vector.tensor_copy(kb[0:64, 0:64], kvcoll[0:64, c0:c0 + 64])
            nc.vector.tensor_copy(kb[64:128, 64:128],
                                  kvcoll[64:128, c0 + 64:c0 + 128])
            kvblk.append(kb)
            kt = sb.tile([128, 2], BF16, tag="ksb", bufs=NP, name=f"ksb{p}")
            nc.gpsimd.memset(kt[:], 0.0)
            nc.vector.tensor_copy(kt[0:64, 0:1],
                                  kvcoll[0:64, c0 + 128:c0 + 129])
            nc.vector.tensor_copy(kt[64:128, 1:2],
                                  kvcoll[64:128, c0 + 128:c0 + 129])
            ksb.append(kt)

        with (
            tc.tile_pool(name="p2sb", bufs=1) as p2,
            tc.tile_pool(name="p2ps", bufs=1, space="PSUM") as ps2,
        ):
            for hv in range(S // S4):
                hb = hv * S4
                xh = [xs_sb[ct][:, hb:hb + S4] for ct in range(KT)]

                dnb = p2.tile([2, NP * S4], F32, tag="dnb", bufs=1,
                              name=f"dnb{hv}")
                qts = []
                for p in range(NP):
                    qt = p2.tile([128, S4], BF16, tag="qt", bufs=NP + 1,
                                 name=f"qt{hv}_{p}")
                    qts.append(qt)
                    qps, t1s, t2s = [], [], []
                    for chk in range(nchk):
                        qp = ps2.tile([128, 512], F32, tag="ps", bufs=6,
                                      name=f"qp{hv}_{p}_{chk}")
                        for ct in range(KT):
                            nc.tensor.matmul(
                                qp[:],
                                wq_sb[ct][:, p * 128:(p + 1) * 128],
                                xh[ct][:, chk * 512:(chk + 1) * 512],
                                start=(ct == 0), stop=(ct == KT - 1))
                        qps.append(qp)
                        t1s.append(p2.tile([128, 512], F32, tag="qt1",
                                           bufs=3,
                                           name=f"qt1_{hv}_{p}_{chk}"))
                        t2s.append(p2.tile([128, 512], F32, tag="qt2",
                                           bufs=3,
                                           name=f"qt2_{hv}_{p}_{chk}"))
                    for chk in range(nchk):
                        qs = qt[:, chk * 512:(chk + 1) * 512]
                        nc.scalar.activation(qs, qps[chk][:], Relu)
                        nc.scalar.activation(t1s[chk][:], qps[chk][:], Relu,
                                             scale=-1.0)
                    for chk in range(nchk):
                        nc.scalar.activation(t2s[chk][:], t1s[chk][:], Exp,
                                             scale=-1.0)
                    for chk in range(nchk):
                        qs = qt[:, chk * 512:(chk + 1) * 512]
                        nc.vector.tensor_add(qs, qs, t2s[chk][:])

                    for chk in range(nchk):
                        dn = ps2.tile([2, 512], F32, tag="dn", bufs=2,
                                      name=f"dn{hv}_{p}_{chk}")
                        nc.tensor.matmul(
                            dn[:], ksb[p][:],
                            qt[:, chk * 512:(chk + 1) * 512],
                            start=True, stop=True)
                        nc.vector.tensor_scalar_max(
                            dnb[:, p * S4 + chk * 512:
                                p * S4 + (chk + 1) * 512], dn[:], 1e-6)

                recb = p2.tile([2, NP * S4], BF16, tag="recb", bufs=1,
                               name=f"recb{hv}")
                with nc.allow_low_precision(reason="recip of clipped denom"):
                    nc.vector.reciprocal(recb[:], dnb[:])

                att = []
                for p in range(NP):
                    qt = qts[p]
                    at = p2.tile([128, S4], BF16, tag="att", bufs=NP + 1,
                                 name=f"att{hv}_{p}")
                    for chk in range(nchk):
                        nm = ps2.tile([128, 512], F32, tag="ps", bufs=6,
                                      name=f"nm{hv}_{p}_{chk}")
                        nc.tensor.matmul(
                            nm[:], kvblk[p][:],
                            qt[:, chk * 512:(chk + 1) * 512],
                            start=True, stop=True)
                        rp = ps2.tile([128, 512], F32, tag="ps", bufs=6,
                                      name=f"rp{hv}_{p}_{chk}")
                        nc.tensor.matmul(
                            rp[:], csel[:],
                            recb[:, p * S4 + chk * 512:
                                 p * S4 + (chk + 1) * 512],
                            start=True, stop=True)
                        ats = at[:, chk * 512:(chk + 1) * 512]
                        nc.scalar.copy(ats, nm[:])
                        nc.vector.tensor_mul(ats, ats, rp[:])
                    att.append(at)

                for mt in range(S4 // 128):
                    r0 = hb + mt * 128
                    for ch in range(2):
                        yp = ps2.tile([128, 512], F32, tag="ps", bufs=6,
                                      name=f"yp{hv}_{mt}_{ch}")
                        for p in range(NP):
                            nc.tensor.matmul(
                                yp[:],
                                att[p][:, mt * 128:(mt + 1) * 128],
                                wo_sb[p][:, ch * 512:(ch + 1) * 512],
                                start=(p == 0), stop=(p == NP - 1))
                        ysb = p2.tile([128, 512], F32, tag="ysb", bufs=3,
                                      name=f"ysb{hv}_{mt}_{ch}")
                        nc.scalar.copy(ysb[:], yp[:])
                        nc.sync.dma_start(
                            out_d[r0:r0 + 128, ch * 512:(ch + 1) * 512],
                            ysb[:])


def _build_bias():
    KT = 9
    KC = KT * 128
    nc = bacc.Bacc("TRN2", target_bir_lowering=False, debug=False,
                   num_devices=N_CORES)
    xt_d = nc.dram_tensor("xt", [KC, S], BF16, kind="ExternalInput").ap()
    wk_d = nc.dram_tensor("wkt", [KC, C], BF16, kind="ExternalInput").ap()
    wv_d = nc.dram_tensor("wvt", [KC, C], BF16, kind="ExternalInput").ap()
    wq_d = nc.dram_tensor("wqt", [KC, C], BF16, kind="ExternalInput").ap()
    wo_d = nc.dram_tensor("wot", [KC, C], BF16, kind="ExternalInput").ap()
    cs_d = nc.dram_tensor("csel", [2, 128], BF16, kind="ExternalInput").ap()
    out_d = nc.dram_tensor("out", [S, C], F32, kind="ExternalOutput").ap()

    with tile.TileContext(nc) as tc:
        _emit_bias(nc, tc, KT, xt_d, wk_d, wv_d, wq_d, wo_d, cs_d, out_d)
    nc.compile()
    return nc


def _prep_host_bias(inputs):
    KT = 9
    KC = KT * 128
    bf = mybir.dt.np(BF16)
    x = np.asarray(inputs["x"], np.float32).reshape(B * T, C)

    def padw(w, b):
        wt = np.ascontiguousarray(np.asarray(w, np.float32).T)
        out = np.zeros((KC, C), np.float32)
        out[:C] = wt
        out[C] = np.asarray(b, np.float32)
        return out.astype(bf)

    wkt = padw(inputs["Wk"], inputs["bk"])
    wvt = padw(inputs["Wv"], inputs["bv"])
    wqt = padw(inputs["Wq"], inputs["bq"])
    wot = padw(inputs["Wo"], np.zeros(C))

    csel = np.zeros((2, 128), np.float32)
    csel[0, :64] = 1.0
    csel[1, 64:] = 1.0
    csel = csel.astype(bf)

    in_maps = []
    for c in range(N_CORES):
        sh = x[c * S:(c + 1) * S]
        xt = np.zeros((KC, S), np.float32)
        xt[:C] = sh.T
        xt[C] = 1.0
        in_maps.append({
            "xt": np.ascontiguousarray(xt.astype(bf)),
            "wkt": wkt, "wvt": wvt, "wqt": wqt, "wot": wot,
            "csel": csel,
        })
    return in_maps


# ---------------------------------------------------------------------------

def _get_nc(has_bias):
    key = has_bias
    if key not in _cache:
        _cache[key] = _build_bias() if has_bias else _build_fp8()
    return _cache[key]


def kernel(**inputs):
    assert np.asarray(inputs["x"]).shape == (B, T, C)
    has_bias = any(
        np.any(np.asarray(inputs[k])) for k in ("bq", "bk", "bv"))
    nc = _get_nc(has_bias)
    in_maps = (_prep_host_bias(inputs) if has_bias
               else _prep_host_fp8(inputs))
    res = bass_utils.run_bass_kernel_spmd(
        nc, in_maps, core_ids=list(range(N_CORES)))
    y = np.concatenate(
        [res.results[c]["out"] for c in range(N_CORES)], axis=0)
    y = y.reshape(B, T, C).astype(np.float32)
    bo = np.asarray(inputs["bo"], np.float32)
    if np.any(bo):
        y = y + bo
    return y


# revision 23
# speedup vs baseline: 1.9316x; 1.0192x over previous
"""Linear attention (ELU+1 feature map) on 8 TRN2 NeuronCores.

Reference math (per batch b):
    q,k,v = x @ W{q,k,v}.T + b;   q,k -> elu(.)+1
    kv[h,d,e] = sum_t k[t,h,d] v[t,h,e];   ks[h,d] = sum_t k[t,h,d]
    out = ((q kv) / clip(q . ks, 1e-6)) @ Wo.T + bo

Sharding: the 16384 tokens are split into 8 contiguous chunks of 2048; core c
owns batch c//2, T-half c%2. kv/ks are partial sums over the core's tokens,
AllReduce-summed within core pairs {0,1},{2,3},{4,5},{6,7}. Everything else is
embarrassingly parallel; the 520 KB pair AllReduce overlaps the q projection.

Fast path (zero projection biases, the common case): the q/k projections run
as fp8(e4m3) DoubleRow matmuls (x1 = fp8(x*SX), W1 = fp8(W.T*SW); the fp8
quantization error of q and k largely cancels between the attention numerator
q.kv and denominator q.ks since both are linear in q and in k -- measured
~9.4e-3 rel err end to end). The v projection needs real precision, so it uses
a 3-term error-compensated fp8 product x1@Wv1 + x1@Wv2 + x2@Wv1 (residuals
quantized at the SAME power-2 scale so all terms share one PSUM accumulation
chain; the dropped x2@Wv2 term is O(fp8_eps^2) ~ bf16 level). DoubleRow packs
two 128-deep contraction tiles per instruction, so q/k cost 1/4 and v 3/4 of
a bf16 projection on the PE. Everything downstream (kv outer products with the
ones-column ksum trick, num/denominator/reciprocal-broadcast matmuls, output
projection) stays bf16.

Engine balance: the elu+1 chain relu(x) + exp(-relu(-x)) is split so no
single engine throttles the PE: relu/exp on ACT, the -relu(-x) on ACT in
phase 1 but DVE in phase 2, the final add on the (otherwise idle) Pool
engine, v unscale-copies and kv aggregation on DVE. kv outer products are
emitted one token-tile behind the projections so the ACT->Pool chain latency
never stalls the PE. Input DMAs are token-chunked and issued on the cheap
gpsimd queue in consumption order so the first matmul starts ~5 us in.

Fallback path (nonzero biases): the original bf16 two-phase kernel with a
ones-row bias fold.
"""

import sys
import numpy as np

for _p in ("/opt/trn_rl_repo", "/opt/pypackages"):
    if _p not in sys.path:
        sys.path.append(_p)

import concourse.bacc as bacc
import concourse.mybir as mybir
import concourse.tile as tile
from concourse import bass_utils

F32 = mybir.dt.float32
BF16 = mybir.dt.bfloat16
FP8 = mybir.dt.float8e4
ACTF = mybir.ActivationFunctionType
DROW = mybir.MatmulPerfMode.DoubleRow

N_CORES = 8
B, T, C = 4, 4096, 1024
H, D = 16, 64
S = B * T // N_CORES          # 2048 tokens per core
NP = 8                        # head pairs (128 channels each)
TT = S // 128                 # 16 token tiles per core
PSTR = 130                    # kv_aug per-pair stride: 128 kv cols + ksum col
                              # + pad col
NJ = 4                        # fp8 stripe pairs (2x128 contraction each)

SX = 2.0                      # fp8 x scale (power of 2)
SW = 512.0                    # fp8 weight scale (power of 2)
US = 1.0 / (SX * SW)          # PSUM unscale

_cache = {}


# ---------------------------------------------------------------------------
# fast path: fp8 DoubleRow projections, zero biases
# ---------------------------------------------------------------------------

def _emit_fp8(nc, tc, x1_d, x2_d, wk1_d, wq1_d, wv1_d, wv2_d, wo_d, cs_d,
              out_d):
    Relu, Exp, Copy = ACTF.Relu, ACTF.Exp, ACTF.Copy
    XCH = 1024                # x DMA token chunk

    with (
        tc.tile_pool(name="sb", bufs=1) as sb,
        tc.tile_pool(name="ps", bufs=1, space="PSUM") as ps,
        tc.tile_pool(name="dram", bufs=1, space="DRAM") as dram,
    ):
        # ---- input DMAs in consumption order on the sync queue: HWDGE
        # descriptor-gen (~0.6 us, off-engine) instead of gpsimd SWDGE
        # (~1.8 us on the Pool engine, which runs the elu adds) ----
        def wtiles(src, tag, eng=None):
            ts = []
            for j in range(NJ):
                w = sb.tile([128, 2 * C], FP8, tag=tag, bufs=NJ,
                            name=f"{tag}{j}")
                (eng or nc.sync).dma_start(w[:], src[j * 128:(j + 1) * 128, :])
                ts.append(w)
            return ts

        wk1 = wtiles(wk1_d, "wk1", eng=nc.gpsimd)

        X1 = [sb.tile([128, 2 * S], FP8, tag="x1", bufs=NJ, name=f"x1_{j}")
              for j in range(NJ)]
        X2 = [sb.tile([128, 2 * S], FP8, tag="x2", bufs=NJ, name=f"x2_{j}")
              for j in range(NJ)]

        def load_x(dst, src_d, c):
            for j in range(NJ):
                for i in range(2):
                    o = i * S + c * XCH
                    nc.sync.dma_start(
                        dst[j][:, o:o + XCH],
                        src_d[j * 128:(j + 1) * 128, o:o + XCH])

        load_x(X1, x1_d, 0)
        wv1 = wtiles(wv1_d, "wv1")
        wv2 = wtiles(wv2_d, "wv2")
        load_x(X2, x2_d, 0)
        load_x(X1, x1_d, 1)
        load_x(X2, x2_d, 1)
        wq1 = wtiles(wq1_d, "wq1")
        wo = []
        for p in range(NP):
            w = sb.tile([128, C], BF16, tag="wo", bufs=NP, name=f"wo{p}")
            nc.sync.dma_start(w[:], wo_d[p * 128:(p + 1) * 128, :])
            wo.append(w)
        csel = sb.tile([2, 128], BF16, tag="csel", name="csel")
        nc.sync.dma_start(csel[:], cs_d[:])

        x1v = [t.rearrange("p (i s) -> p i s", i=2) for t in X1]
        x2v = [t.rearrange("p (i s) -> p i s", i=2) for t in X2]
        wk1v = [t.rearrange("p (i s) -> p i s", i=2) for t in wk1]
        wq1v = [t.rearrange("p (i s) -> p i s", i=2) for t in wq1]
        wv1v = [t.rearrange("p (i s) -> p i s", i=2) for t in wv1]
        wv2v = [t.rearrange("p (i s) -> p i s", i=2) for t in wv2]

        kvagg = sb.tile([128, NP * PSTR], F32, tag="kvagg", name="kvagg")
        nc.gpsimd.memset(kvagg[:], 0.0)

        # ------------- phase 1: k/v projections + kv aggregation -----------
        # kv matmuls run one token-tile behind the projections so the
        # ACT->Pool elu chain latency never stalls the PE.
        pend = None

        def flush_kv(ktok, vaug, tt):
            for g in range(3):
                p0, p1n = 3 * g, min(3 * g + 3, NP)
                kvt = ps.tile([128, (p1n - p0) * PSTR], F32, tag="qps",
                              bufs=2, name=f"kvt{tt}_{g}",
                              padded_shape=[128, 512])
                for p in range(p0, p1n):
                    j = p - p0
                    nc.tensor.matmul(
                        kvt[:, j * PSTR:(j + 1) * PSTR],
                        ktok[:, p * 128:(p + 1) * 128],
                        vaug[:, p * PSTR:(p + 1) * PSTR],
                        start=True, stop=True)
                nc.vector.tensor_add(
                    kvagg[:, p0 * PSTR:p1n * PSTR],
                    kvagg[:, p0 * PSTR:p1n * PSTR], kvt[:])

        for tt in range(TT):
            t0 = tt * 128
            ktok = sb.tile([128, C], BF16, tag="ktok", bufs=5,
                           name=f"ktok{tt}")
            t1 = sb.tile([128, C], BF16, tag="t1", bufs=5, name=f"t1_{tt}")
            t2 = sb.tile([128, C], BF16, tag="t2", bufs=5, name=f"t2_{tt}")
            for ch in range(2):
                kp = ps.tile([128, 512], F32, tag="ps", bufs=6,
                             name=f"kp{tt}_{ch}")
                for j in range(NJ):
                    nc.tensor.matmul(
                        kp[:],
                        x1v[j][:, :, t0:t0 + 128],
                        wk1v[j][:, :, ch * 512:(ch + 1) * 512],
                        start=(j == 0), stop=(j == NJ - 1), perf_mode=DROW)
                cs = slice(ch * 512, (ch + 1) * 512)
                nc.scalar.activation(ktok[:, cs], kp[:], Relu, scale=US)
                nc.vector.tensor_scalar(t1[:, cs], kp[:], -US, 0.0,
                                        mybir.AluOpType.mult,
                                        mybir.AluOpType.max)
                nc.scalar.activation(t2[:, cs], t1[:, cs], Exp, scale=-1.0)
                nc.gpsimd.tensor_add(ktok[:, cs], ktok[:, cs], t2[:, cs])

            vaug = sb.tile([128, NP * PSTR], BF16, tag="vaug", bufs=5,
                           name=f"vaug{tt}")
            nc.gpsimd.memset(vaug[:], 1.0)
            vau3 = vaug.rearrange("p (g c) -> p g c", c=PSTR)
            for ch in range(2):
                vp = ps.tile([128, 512], F32, tag="ps", bufs=6,
                             name=f"vp{tt}_{ch}")
                n = 0
                for xv, wv in ((x1v, wv1v), (x1v, wv2v), (x2v, wv1v)):
                    for j in range(NJ):
                        nc.tensor.matmul(
                            vp[:],
                            xv[j][:, :, t0:t0 + 128],
                            wv[j][:, :, ch * 512:(ch + 1) * 512],
                            start=(n == 0), stop=(n == 3 * NJ - 1),
                            perf_mode=DROW)
                        n += 1
                dst = vau3[:, ch * 4:(ch + 1) * 4, 0:128]
                src = vp[:].rearrange("p (g c) -> p g c", c=128)
                nc.scalar.activation(dst, src, Copy, scale=US)

            if pend is not None:
                flush_kv(*pend)
            pend = (ktok, vaug, tt)
        flush_kv(*pend)

        # ------------- pair AllReduce --------------------------------------
        bounce_in = dram.tile([128, NP * PSTR], F32, name="bounce_in")
        bounce_out = dram.tile([128, NP * PSTR], F32, name="bounce_out")
        nc.sync.dma_start(bounce_in[:], kvagg[:])
        nc.gpsimd.collective_compute(
            "AllReduce", mybir.AluOpType.add,
            ins=[bounce_in.opt()], outs=[bounce_out.opt()],
            replica_groups=[[2 * i, 2 * i + 1] for i in range(N_CORES // 2)])
        kvcoll = sb.tile([128, NP * PSTR], F32, tag="kvcoll", name="kvcoll")
        nc.sync.dma_start(kvcoll[:], bounce_out[:])

        # fp8c2 quantization of the collective result, batched across all
        # pairs with strided 3D APs.  kvb12 packs [kvb1 | kvb2] per pair
        # (block-diag zeroed), ksb12 packs ksum main+residual columns at a
        # 16-byte step for the DoubleRow weight AP.  Scale KS cancels
        # exactly between numerator and 1/denominator.
        KS = 1.0 / 32.0
        Sub, Mult = mybir.AluOpType.subtract, mybir.AluOpType.mult
        kvb12 = sb.tile([128, NP * 256], FP8, tag="kvb12", name="kvb12")
        ksb12 = sb.tile([128, NP * 32], FP8, tag="ksb12", name="ksb12")
        nc.gpsimd.memset(kvb12[:], 0.0)
        nc.gpsimd.memset(ksb12[:], 0.0)
        kc3 = kvcoll.rearrange("p (g c) -> p g c", c=PSTR)
        kb3 = kvb12.rearrange("p (g c) -> p g c", c=256)
        ks3 = ksb12.rearrange("p (g c) -> p g c", c=32)
        # ksum main + residual (tiny ops; dn consumes these first)
        nc.scalar.activation(ks3[0:64, :, 0:1], kc3[0:64, :, 128:129],
                             ACTF.Copy, scale=KS)
        nc.scalar.activation(ks3[64:128, :, 1:2], kc3[64:128, :, 128:129],
                             ACTF.Copy, scale=KS)
        nc.vector.scalar_tensor_tensor(
            ks3[0:64, :, 16:17], kc3[0:64, :, 128:129], KS,
            ks3[0:64, :, 0:1], Mult, Sub)
        nc.vector.scalar_tensor_tensor(
            ks3[64:128, :, 17:18], kc3[64:128, :, 128:129], KS,
            ks3[64:128, :, 1:2], Mult, Sub)
        # kv blocks main + residual
        nc.scalar.activation(kb3[0:64, :, 0:64], kc3[0:64, :, 0:64],
                             ACTF.Copy, scale=KS)
        nc.scalar.activation(kb3[64:128, :, 64:128], kc3[64:128, :, 64:128],
                             ACTF.Copy, scale=KS)
        nc.vector.scalar_tensor_tensor(
            kb3[0:64, :, 128:192], kc3[0:64, :, 0:64], KS,
            kb3[0:64, :, 0:64], Mult, Sub)
        nc.vector.scalar_tensor_tensor(
            kb3[64:128, :, 192:256], kc3[64:128, :, 64:128], KS,
            kb3[64:128, :, 64:128], Mult, Sub)

        # ------------- phase 2: q, attention, output projection ------------
        # Section order: q(hv0) | norm(hv0) | q(hv1) | out(hv0) | norm(hv1)
        # | out(hv1).  The engines run their queues in order, so hv1's
        # ACT-heavy q chains must NOT sit between hv0's norm and hv0's
        # ACT-light output projection; interleaving this way keeps every
        # engine's queue aligned with the PE's needs.  q(hv0) also covers
        # the AllReduce + kvblk/ksb prep latency.
        HSPAN = 1024
        NHV = S // HSPAN

        def q_chains(hv):
            hb = hv * HSPAN
            qts = []
            for p in range(NP):
                qt = sb.tile([128, HSPAN], FP8, tag="qt", bufs=NP + 2,
                             name=f"qt{hv}_{p}")
                t1 = sb.tile([128, HSPAN], BF16, tag="t1", bufs=5,
                             name=f"qt1_{hv}_{p}")
                t2 = sb.tile([128, HSPAN], BF16, tag="t2", bufs=5,
                             name=f"qt2_{hv}_{p}")
                for chk in range(2):
                    c0 = hb + chk * 512
                    qp = ps.tile([128, 512], F32, tag="qps", bufs=2,
                                 name=f"qp{hv}_{p}_{chk}")
                    for j in range(NJ):
                        nc.tensor.matmul(
                            qp[:],
                            wq1v[j][:, :, p * 128:(p + 1) * 128],
                            x1v[j][:, :, c0:c0 + 512],
                            start=(j == 0), stop=(j == NJ - 1),
                            perf_mode=DROW)
                    cs = slice(chk * 512, (chk + 1) * 512)
                    nc.scalar.activation(qt[:, cs], qp[:], Relu, scale=US)
                    nc.vector.tensor_scalar(t1[:, cs], qp[:], -US, 0.0,
                                            mybir.AluOpType.mult,
                                            mybir.AluOpType.max)
                    nc.scalar.activation(t2[:, cs], t1[:, cs], Exp,
                                         scale=-1.0)
                    nc.gpsimd.tensor_add(qt[:, cs], qt[:, cs], t2[:, cs])
                qts.append(qt)
            return qts

        def norm_section(hv, qts):
            # The PE runs strictly in order, so a reciprocal-blocked rp
            # matmul would stall every later matmul behind it.  Wave order:
            # all denominators (reciprocals compute on DVE meanwhile), then
            # all numerators, then all rp broadcasts — by the time the first
            # rp issues, its reciprocal is long done.
            att = []
            recbs = []
            for p in range(NP):
                qt = qts[p]
                recb = sb.tile([2, HSPAN], BF16, tag="recb", bufs=NP + 1,
                               name=f"recb{hv}_{p}")
                ksw = ksb12[:, p * 32:(p + 1) * 32].rearrange(
                    "p (i c) -> p i c", c=16)[:, :, 0:2]
                for chk in range(2):
                    dn = ps.tile([2, 512], F32, tag="ps", bufs=6,
                                 name=f"dn{hv}_{p}_{chk}")
                    qb = qt[:, chk * 512:(chk + 1) * 512].unsqueeze(
                        1).broadcast_to((128, 2, 512))
                    nc.tensor.matmul(dn[:], ksw, qb,
                                     start=True, stop=True, perf_mode=DROW)
                    # denom = q.ks with q,ks > 0 elementwise (elu+1), so the
                    # reference clip(1e-6) is unreachable; invert directly.
                    with nc.allow_low_precision(reason="recip of pos denom"):
                        nc.vector.reciprocal(
                            recb[:, chk * 512:(chk + 1) * 512], dn[:])
                recbs.append(recb)

            for p in range(NP):
                at = sb.tile([128, HSPAN], BF16, tag="att", bufs=NP + 2,
                             name=f"att{hv}_{p}")
                for chk in range(2):
                    cs = slice(chk * 512, (chk + 1) * 512)
                    nm = ps.tile([128, 512], F32, tag="ps", bufs=6,
                                 name=f"nm{hv}_{p}_{chk}")
                    kvw = kvb12[:, p * 256:(p + 1) * 256].rearrange(
                        "p (i c) -> p i c", c=128)
                    qb = qts[p][:, cs].unsqueeze(1).broadcast_to(
                        (128, 2, 512))
                    nc.tensor.matmul(nm[:], kvw, qb,
                                     start=True, stop=True, perf_mode=DROW)
                    # DVE can read only one PSUM operand: stage nm via ACT
                    nc.scalar.activation(at[:, cs], nm[:], Copy)
                att.append(at)

            for p in range(NP):
                for chk in range(2):
                    cs = slice(chk * 512, (chk + 1) * 512)
                    rp = ps.tile([128, 512], F32, tag="ps", bufs=6,
                                 name=f"rp{hv}_{p}_{chk}")
                    nc.tensor.matmul(rp[:], csel[:], recbs[p][:, cs],
                                     start=True, stop=True)
                    nc.vector.tensor_mul(att[p][:, cs], att[p][:, cs],
                                         rp[:])
            return att

        def out_section(hv, att):
            hb = hv * HSPAN
            for mt in range(HSPAN // 128):
                r0 = hb + mt * 128
                ysb = sb.tile([128, 1024], F32, tag="ysb", bufs=3,
                              name=f"ysb{hv}_{mt}")
                for ch in range(2):
                    yp = ps.tile([128, 512], F32, tag="ps", bufs=6,
                                 name=f"yp{hv}_{mt}_{ch}")
                    for p in range(NP):
                        nc.tensor.matmul(
                            yp[:],
                            att[p][:, mt * 128:(mt + 1) * 128],
                            wo[p][:, ch * 512:(ch + 1) * 512],
                            start=(p == 0), stop=(p == NP - 1))
                    # split ysb staging across ACT and DVE
                    if ch == 0:
                        nc.scalar.activation(
                            ysb[:, ch * 512:(ch + 1) * 512], yp[:], Copy)
                    else:
                        nc.vector.tensor_copy(
                            ysb[:, ch * 512:(ch + 1) * 512], yp[:])
                # alternate out DMAs between the gpsimd queue (SWDGE on the
                # tail-idle Pool engine) and the sync queue (HWDGE, idle
                # after the input loads) to halve per-queue serialization.
                eng = nc.gpsimd if mt % 2 == 0 else nc.sync
                eng.dma_start(out_d[r0:r0 + 128, :], ysb[:])

        q0 = q_chains(0)
        att0 = norm_section(0, q0)
        q1 = q_chains(1)
        out_section(0, att0)
        att1 = norm_section(1, q1)
        out_section(1, att1)


def _build_fp8():
    nc = bacc.Bacc("TRN2", target_bir_lowering=False, debug=False,
                   num_devices=N_CORES)
    x1_d = nc.dram_tensor("x1", [NJ * 128, 2 * S], FP8,
                          kind="ExternalInput").ap()
    x2_d = nc.dram_tensor("x2", [NJ * 128, 2 * S], FP8,
                          kind="ExternalInput").ap()
    wk1_d = nc.dram_tensor("wk1", [NJ * 128, 2 * C], FP8,
                           kind="ExternalInput").ap()
    wq1_d = nc.dram_tensor("wq1", [NJ * 128, 2 * C], FP8,
                           kind="ExternalInput").ap()
    wv1_d = nc.dram_tensor("wv1", [NJ * 128, 2 * C], FP8,
                           kind="ExternalInput").ap()
    wv2_d = nc.dram_tensor("wv2", [NJ * 128, 2 * C], FP8,
                           kind="ExternalInput").ap()
    wo_d = nc.dram_tensor("wo", [C, C], BF16, kind="ExternalInput").ap()
    cs_d = nc.dram_tensor("csel", [2, 128], BF16, kind="ExternalInput").ap()
    out_d = nc.dram_tensor("out", [S, C], F32, kind="ExternalOutput").ap()

    with tile.TileContext(nc) as tc:
        _emit_fp8(nc, tc, x1_d, x2_d, wk1_d, wq1_d, wv1_d, wv2_d, wo_d,
                  cs_d, out_d)
    nc.compile()
    return nc


def _pack_pairs(a):
    """[C, N] -> stripe-pair packed [NJ*128, 2*N] (j, r, i, t)."""
    Cin, N = a.shape
    return np.ascontiguousarray(
        a.reshape(NJ, 2, 128, N).transpose(0, 2, 1, 3).reshape(NJ * 128,
                                                               2 * N))


def _prep_host_fp8(inputs):
    f8 = mybir.dt.np(FP8)
    bf = mybir.dt.np(BF16)
    x = np.asarray(inputs["x"], np.float32).reshape(B * T, C)

    def q8(a):
        return (a * SW).astype(f8)

    wk1 = _pack_pairs(q8(np.ascontiguousarray(
        np.asarray(inputs["Wk"], np.float32).T)))
    wq1 = _pack_pairs(q8(np.ascontiguousarray(
        np.asarray(inputs["Wq"], np.float32).T)))
    wvt = np.ascontiguousarray(np.asarray(inputs["Wv"], np.float32).T)
    wv1q = q8(wvt)
    wv1 = _pack_pairs(wv1q)
    wv2 = _pack_pairs(q8(wvt - wv1q.astype(np.float32) / SW))
    wot = np.ascontiguousarray(
        np.asarray(inputs["Wo"], np.float32).T).astype(bf)

    csel = np.zeros((2, 128), np.float32)
    csel[0, :64] = 1.0
    csel[1, 64:] = 1.0
    csel = csel.astype(bf)

    in_maps = []
    for c in range(N_CORES):
        sh = np.ascontiguousarray(x[c * S:(c + 1) * S].T)  # [C, S]
        x1q = (sh * SX).astype(f8)
        x2q = ((sh - x1q.astype(np.float32) / SX) * SX).astype(f8)
        in_maps.append({
            "x1": _pack_pairs(x1q),
            "x2": _pack_pairs(x2q),
            "wk1": wk1, "wq1": wq1, "wv1": wv1, "wv2": wv2,
            "wo": wot, "csel": csel,
        })
    return in_maps


# ---------------------------------------------------------------------------
# fallback path (nonzero biases): original bf16 two-phase kernel
# ---------------------------------------------------------------------------

def _emit_bias(nc, tc, KT, xt_d, wk_d, wv_d, wq_d, wo_d, cs_d, out_d):
    S4 = S // 2
    nchk = S4 // 512
    Relu, Exp = ACTF.Relu, ACTF.Exp
    WB = 2 * KT

    with (
        tc.tile_pool(name="wpool", bufs=1) as wpool,
        tc.tile_pool(name="persist", bufs=1) as sb,
        tc.tile_pool(name="dram", bufs=1, space="DRAM") as dram,
    ):
        wk_sb = []
        wv_sb = []
        for ct in range(KT):
            w = wpool.tile([128, C], BF16, tag="w", bufs=WB, name=f"wk{ct}")
            nc.gpsimd.dma_start(w[:], wk_d[ct * 128:(ct + 1) * 128, :])
            wk_sb.append(w)
        for ct in range(KT):
            w = wpool.tile([128, C], BF16, tag="w", bufs=WB, name=f"wv{ct}")
            nc.gpsimd.dma_start(w[:], wv_d[ct * 128:(ct + 1) * 128, :])
            wv_sb.append(w)

        csel = sb.tile([2, 128], BF16, tag="csel", name="csel")
        nc.sync.dma_start(csel[:], cs_d[:])

        kvagg = sb.tile([128, NP * PSTR], F32, tag="kvagg", name="kvagg")

        with (
            tc.tile_pool(name="p1sb", bufs=1) as p1,
            tc.tile_pool(name="p1ps", bufs=1, space="PSUM") as ps1,
        ):
            nc.gpsimd.memset(kvagg[:], 0.0)

            xs_sb = []
            for ct in range(KT):
                xst = sb.tile([128, S], BF16, tag="xs", bufs=KT,
                              name=f"xs{ct}")
                nc.gpsimd.dma_start(xst[:], xt_d[ct * 128:(ct + 1) * 128, :])
                xs_sb.append(xst)

            for tt in range(TT):
                t0 = tt * 128
                xb = [xs_sb[ct][:, t0:t0 + 128] for ct in range(KT)]

                ktok = p1.tile([128, C], BF16, tag="ktok", bufs=3,
                               name=f"ktok{tt}")
                kps, t1s, t2s = [], [], []
                for ch in range(2):
                    kp = ps1.tile([128, 512], F32, tag="ps", bufs=4,
                                  name=f"kp{tt}_{ch}")
                    for ct in range(KT):
                        nc.tensor.matmul(
                            kp[:], xb[ct],
                            wk_sb[ct][:, ch * 512:(ch + 1) * 512],
                            start=(ct == 0), stop=(ct == KT - 1))
                    kps.append(kp)
                    t1s.append(p1.tile([128, 512], F32, tag="t1", bufs=3,
                                       name=f"t1_{tt}_{ch}"))
                    t2s.append(p1.tile([128, 512], F32, tag="t2", bufs=3,
                                       name=f"t2_{tt}_{ch}"))
                for ch in range(2):
                    ks = ktok[:, ch * 512:(ch + 1) * 512]
                    nc.scalar.activation(ks, kps[ch][:], Relu)
                    nc.scalar.activation(t1s[ch][:], kps[ch][:], Relu,
                                         scale=-1.0)
                for ch in range(2):
                    nc.scalar.activation(t2s[ch][:], t1s[ch][:], Exp,
                                         scale=-1.0)
                for ch in range(2):
                    ks = ktok[:, ch * 512:(ch + 1) * 512]
                    nc.vector.tensor_add(ks, ks, t2s[ch][:])

                vaug = p1.tile([128, NP * PSTR], BF16, tag="vaug", bufs=3,
                               name=f"vaug{tt}")
                nc.gpsimd.memset(vaug[:], 1.0)
                vau3 = vaug.rearrange("p (g c) -> p g c", c=PSTR)
                for ch in range(2):
                    vp = ps1.tile([128, 512], F32, tag="ps", bufs=4,
                                  name=f"vp{tt}_{ch}")
                    for ct in range(KT):
                        nc.tensor.matmul(
                            vp[:], xb[ct],
                            wv_sb[ct][:, ch * 512:(ch + 1) * 512],
                            start=(ct == 0), stop=(ct == KT - 1))
                    nc.vector.tensor_copy(
                        vau3[:, ch * 4:(ch + 1) * 4, 0:128],
                        vp[:].rearrange("p (g c) -> p g c", c=128))

                for g in range(3):
                    p0, p1n = 3 * g, min(3 * g + 3, NP)
                    kvt = ps1.tile([128, (p1n - p0) * PSTR], F32, tag="kvt",
                                   bufs=3, name=f"kvt{tt}_{g}",
                                   padded_shape=[128, 3 * PSTR])
                    for p in range(p0, p1n):
                        j = p - p0
                        nc.tensor.matmul(
                            kvt[:, j * PSTR:(j + 1) * PSTR],
                            ktok[:, p * 128:(p + 1) * 128],
                            vaug[:, p * PSTR:(p + 1) * PSTR],
                            start=True, stop=True)
                    nc.vector.tensor_add(
                        kvagg[:, p0 * PSTR:p1n * PSTR],
                        kvagg[:, p0 * PSTR:p1n * PSTR], kvt[:])

        bounce_in = dram.tile([128, NP * PSTR], F32, name="bounce_in")
        bounce_out = dram.tile([128, NP * PSTR], F32, name="bounce_out")
        nc.sync.dma_start(bounce_in[:], kvagg[:])
        nc.gpsimd.collective_compute(
            "AllReduce", mybir.AluOpType.add,
            ins=[bounce_in.opt()], outs=[bounce_out.opt()],
            replica_groups=[[2 * i, 2 * i + 1] for i in range(N_CORES // 2)])
        kvcoll = sb.tile([128, NP * PSTR], F32, tag="kvcoll", name="kvcoll")
        nc.sync.dma_start(kvcoll[:], bounce_out[:])

        wq_sb = []
        wo_sb = []
        for ct in range(KT):
            w = wpool.tile([128, C], BF16, tag="w", bufs=WB, name=f"wq{ct}")
            nc.gpsimd.dma_start(w[:], wq_d[ct * 128:(ct + 1) * 128, :])
            wq_sb.append(w)
        for ct in range(NP):
            w = wpool.tile([128, C], BF16, tag="w", bufs=WB, name=f"wo{ct}")
            nc.gpsimd.dma_start(w[:], wo_d[ct * 128:(ct + 1) * 128, :])
            wo_sb.append(w)

        kvblk = []
        ksb = []
        for p in range(NP):
            c0 = p * PSTR
            kb = sb.tile([128, 128], BF16, tag="kvblk", bufs=NP,
                         name=f"kvblk{p}")
            nc.gpsimd.memset(kb[:], 0.0)
            nc.vector.tensor_copy(kb[0:64, 0:64], kvcoll[0:64, c0:c0 + 64])
            nc.vector.tensor_copy(kb[64:128, 64:128],
                                  kvcoll[64:128, c0 + 64:c0 + 128])
            kvblk.append(kb)
            kt = sb.tile([128, 2], BF16, tag="ksb", bufs=NP, name=f"ksb{p}")
            nc.gpsimd.memset(kt[:], 0.0)
            nc.vector.tensor_copy(kt[0:64, 0:1],
                                  kvcoll[0:64, c0 + 128:c0 + 129])
            nc.vector.tensor_copy(kt[64:128, 1:2],
                                  kvcoll[64:128, c0 + 128:c0 + 129])
            ksb.append(kt)

        with (
            tc.tile_pool(name="p2sb", bufs=1) as p2,
            tc.tile_pool(name="p2ps", bufs=1, space="PSUM") as ps2,
        ):
            for hv in range(S // S4):
                hb = hv * S4
                xh = [xs_sb[ct][:, hb:hb + S4] for ct in range(KT)]

                dnb = p2.tile([2, NP * S4], F32, tag="dnb", bufs=1,
                              name=f"dnb{hv}")
                qts = []
                for p in range(NP):
                    qt = p2.tile([128, S4], BF16, tag="qt", bufs=NP + 1,
                                 name=f"qt{hv}_{p}")
                    qts.append(qt)
                    qps, t1s, t2s = [], [], []
                    for chk in range(nchk):
                        qp = ps2.tile([128, 512], F32, tag="ps", bufs=6,
                                      name=f"qp{hv}_{p}_{chk}")
                        for ct in range(KT):
                            nc.tensor.matmul(
                                qp[:],
                                wq_sb[ct][:, p * 128:(p + 1) * 128],
                                xh[ct][:, chk * 512:(chk + 1) * 512],
                                start=(ct == 0), stop=(ct == KT - 1))
                        qps.append(qp)
                        t1s.append(p2.tile([128, 512], F32, tag="qt1",
                                           bufs=3,
                                           name=f"qt1_{hv}_{p}_{chk}"))
                        t2s.append(p2.tile([128, 512], F32, tag="qt2",
                                           bufs=3,
                                           name=f"qt2_{hv}_{p}_{chk}"))
                    for chk in range(nchk):
                        qs = qt[:, chk * 512:(chk + 1) * 512]
                        nc.scalar.activation(qs, qps[chk][:], Relu)
                        nc.scalar.activation(t1s[chk][:], qps[chk][:], Relu,
                                             scale=-1.0)
                    for chk in range(nchk):
                        nc.scalar.activation(t2s[chk][:], t1s[chk][:], Exp,
                                             scale=-1.0)
                    for chk in range(nchk):
                        qs = qt[:, chk * 512:(chk + 1) * 512]
                        nc.vector.tensor_add(qs, qs, t2s[chk][:])

                    for chk in range(nchk):
                        dn = ps2.tile([2, 512], F32, tag="dn", bufs=2,
                                      name=f"dn{hv}_{p}_{chk}")
                        nc.tensor.matmul(
                            dn[:], ksb[p][:],
                            qt[:, chk * 512:(chk + 1) * 512],
                            start=True, stop=True)
                        nc.vector.tensor_scalar_max(
                            dnb[:, p * S4 + chk * 512:
                                p * S4 + (chk + 1) * 512], dn[:], 1e-6)

                recb = p2.tile([2, NP * S4], BF16, tag="recb", bufs=1,
                               name=f"recb{hv}")
                with nc.allow_low_precision(reason="recip of clipped denom"):
                    nc.vector.reciprocal(recb[:], dnb[:])

                att = []
                for p in range(NP):
                    qt = qts[p]
                    at = p2.tile([128, S4], BF16, tag="att", bufs=NP + 1,
                                 name=f"att{hv}_{p}")
                    for chk in range(nchk):
                        nm = ps2.tile([128, 512], F32, tag="ps", bufs=6,
                                      name=f"nm{hv}_{p}_{chk}")
                        nc.tensor.matmul(
                            nm[:], kvblk[p][:],
                            qt[:, chk * 512:(chk + 1) * 512],
                            start=True, stop=True)
                        rp = ps2.tile([128, 512], F32, tag="ps", bufs=6,
                                      name=f"rp{hv}_{p}_{chk}")
                        nc.tensor.matmul(
                            rp[:], csel[:],
                            recb[:, p * S4 + chk * 512:
                                 p * S4 + (chk + 1) * 512],
                            start=True, stop=True)
                        ats = at[:, chk * 512:(chk + 1) * 512]
                        nc.scalar.copy(ats, nm[:])
                        nc.vector.tensor_mul(ats, ats, rp[:])
                    att.append(at)

                for mt in range(S4 // 128):
                    r0 = hb + mt * 128
                    for ch in range(2):
                        yp = ps2.tile([128, 512], F32, tag="ps", bufs=6,
                                      name=f"yp{hv}_{mt}_{ch}")
                        for p in range(NP):
                            nc.tensor.matmul(
                                yp[:],
                                att[p][:, mt * 128:(mt + 1) * 128],
                                wo_sb[p][:, ch * 512:(ch + 1) * 512],
                                start=(p == 0), stop=(p == NP - 1))
                        ysb = p2.tile([128, 512], F32, tag="ysb", bufs=3,
                                      name=f"ysb{hv}_{mt}_{ch}")
                        nc.scalar.copy(ysb[:], yp[:])
                        nc.sync.dma_start(
                            out_d[r0:r0 + 128, ch * 512:(ch + 1) * 512],
                            ysb[:])


def _build_bias():
    KT = 9
    KC = KT * 128
    nc = bacc.Bacc("TRN2", target_bir_lowering=False, debug=False,
                   num_devices=N_CORES)
    xt_d = nc.dram_tensor("xt", [KC, S], BF16, kind="ExternalInput").ap()
    wk_d = nc.dram_tensor("wkt", [KC, C], BF16, kind="ExternalInput").ap()
    wv_d = nc.dram_tensor("wvt", [KC, C], BF16, kind="ExternalInput").ap()
    wq_d = nc.dram_tensor("wqt", [KC, C], BF16, kind="ExternalInput").ap()
    wo_d = nc.dram_tensor("wot", [KC, C], BF16, kind="ExternalInput").ap()
    cs_d = nc.dram_tensor("csel", [2, 128], BF16, kind="ExternalInput").ap()
    out_d = nc.dram_tensor("out", [S, C], F32, kind="ExternalOutput").ap()

    with tile.TileContext(nc) as tc:
        _emit_bias(nc, tc, KT, xt_d, wk_d, wv_d, wq_d, wo_d, cs_d, out_d)
    nc.compile()
    return nc


def _prep_host_bias(inputs):
    KT = 9
    KC = KT * 128
    bf = mybir.dt.np(BF16)
    x = np.asarray(inputs["x"], np.float32).reshape(B * T, C)

    def padw(w, b):
        wt = np.ascontiguousarray(np.asarray(w, np.float32).T)
        out = np.zeros((KC, C), np.float32)
        out[:C] = wt
        out[C] = np.asarray(b, np.float32)
        return out.astype(bf)

    wkt = padw(inputs["Wk"], inputs["bk"])
    wvt = padw(inputs["Wv"], inputs["bv"])
    wqt = padw(inputs["Wq"], inputs["bq"])
    wot = padw(inputs["Wo"], np.zeros(C))

    csel = np.zeros((2, 128), np.float32)
    csel[0, :64] = 1.0
    csel[1, 64:] = 1.0
    csel = csel.astype(bf)

    in_maps = []
    for c in range(N_CORES):
        sh = x[c * S:(c + 1) * S]
        xt = np.zeros((KC, S), np.float32)
        xt[:C] = sh.T
        xt[C] = 1.0
        in_maps.append({
            "xt": np.ascontiguousarray(xt.astype(bf)),
            "wkt": wkt, "wvt": wvt, "wqt": wqt, "wot": wot,
            "csel": csel,
        })
    return in_maps


# ---------------------------------------------------------------------------

def _get_nc(has_bias):
    key = has_bias
    if key not in _cache:
        _cache[key] = _build_bias() if has_bias else _build_fp8()
    return _cache[key]


def kernel(**inputs):
    assert np.asarray(inputs["x"]).shape == (B, T, C)
    has_bias = any(
        np.any(np.asarray(inputs[k])) for k in ("bq", "bk", "bv"))
    nc = _get_nc(has_bias)
    in_maps = (_prep_host_bias(inputs) if has_bias
               else _prep_host_fp8(inputs))
    res = bass_utils.run_bass_kernel_spmd(
        nc, in_maps, core_ids=list(range(N_CORES)))
    y = np.concatenate(
        [res.results[c]["out"] for c in range(N_CORES)], axis=0)
    y = y.reshape(B, T, C).astype(np.float32)
    bo = np.asarray(inputs["bo"], np.float32)
    if np.any(bo):
        y = y + bo
    return y
